# revision 13
# baseline (speedup 1.0000x reference)
"""Trainium2 Bass kernel for nn_Circuit_19275813225041.

24-qubit state-vector simulation: one layer of single-qubit gates on every
qubit, then a ladder of two-qubit gates on neighboring pairs (q, q+1),
q = 0..22, on a 2^24 complex state stored as (2, 2^24) float32 (re, im).

Strategy (8 NeuronCores):
  - Qubit q <-> bit q of the state index, bit 0 = MSB.
  - Shard the state over the 3 LSB qubits (q21,q22,q23): core d holds
    amplitudes with index % 8 == d (state-vector slicing).
  - Gates are fused on the host into 4 big chunk matrices:
      U1: 128x128 on qubits [0..6]    (singles 0..6, ladder (0,1)..(5,6))
      U2: 256x256 on qubits [6..13]   (singles 7..13, ladder (6,7)..(12,13))
      U3: 256x256 on qubits [13..20]  (singles 14..20, ladder (13,14)..(19,20))
      U4: 128x128 on qubits [21,22,23,17..20] (singles 21..23, ladder
          (20,21),(21,22),(22,23), identity on q17..q19)
    (The reference's _apply_gate has a permutation quirk for the 2-qubit
    gate at q=1 -- its "inverse" transpose applies perm again, which for
    q=1 is a 3-cycle.  This adds a relabeling permutation on qubits
    (0,1,2) right after that gate; it is folded into U1.)
  - Each core applies U1..U3 to its local 2^21 state via TensorE matmuls,
    with PE transposes rotating 7-bit groups through the partition axis and
    2-term PSUM accumulation handling the chunk boundary bit (q6, q13).
  - One AllToAll swaps qubits (q0,q1,q2) <-> (q21,q22,q23) across cores so
    the final chunk U4 applies locally; output is returned sharded over
    (q0,q1,q2) and reassembled on the host.
"""

import numpy as np

import concourse.bass as bass
import concourse.bacc as bacc
import concourse.mybir as mybir
import concourse.tile as tile
from concourse.bass_utils import run_bass_kernel_spmd

F32 = mybir.dt.float32
F32R = mybir.dt.float32r

USE_F32R = True   # fast fp32 matmul mode (reduced precision)
N_CORES = 8


# ---------------------------------------------------------------------------
# Host-side gate fusion
# ---------------------------------------------------------------------------

def _embed_gate(mat, qubits, group):
    """Embed `mat` acting on `qubits` (MSB-first) into the space indexed by
    `group` (list of qubits, group[0] = MSB of the index)."""
    g = len(group)
    k = len(qubits)
    pos = [group.index(q) for q in qubits]
    rest = [i for i in range(g) if i not in pos]
    U = np.zeros((1 << g, 1 << g), dtype=np.complex128)
    for r in range(1 << len(rest)):
        base = 0
        for bi, p in enumerate(rest):
            if (r >> (len(rest) - 1 - bi)) & 1:
                base |= 1 << (g - 1 - p)
        for a in range(1 << k):
            ia = base
            for bi, p in enumerate(pos):
                if (a >> (k - 1 - bi)) & 1:
                    ia |= 1 << (g - 1 - p)
            for b in range(1 << k):
                ib = base
                for bi, p in enumerate(pos):
                    if (b >> (k - 1 - bi)) & 1:
                        ib |= 1 << (g - 1 - p)
                U[ia, ib] = mat[a, b]
    return U


def _quirk_P():
    # reference._apply_gate on [1,2]: the un-permute uses perm (a 3-cycle)
    # instead of its inverse => extra relabeling on qubits (0,1,2):
    # new (b0,b1,b2) = (old b2, old b0, old b1).
    P = np.zeros((8, 8), dtype=np.complex128)
    for b0 in range(2):
        for b1 in range(2):
            for b2 in range(2):
                P[(b2 << 2) | (b0 << 1) | b1, (b0 << 2) | (b1 << 1) | b2] = 1
    return P


def _fuse(ops, group):
    U = np.eye(1 << len(group), dtype=np.complex128)
    for mat, qb in ops:
        U = _embed_gate(mat, qb, group) @ U
    return U


def build_chunk_matrices(gates1, gates2):
    g1 = gates1[:, 0].astype(np.float64) + 1j * gates1[:, 1].astype(np.float64)
    g2 = gates2[:, 0].astype(np.float64) + 1j * gates2[:, 1].astype(np.float64)

    ops1 = [(g1[q], [q]) for q in range(0, 7)]
    ops1 += [(g2[0], [0, 1]), (g2[1], [1, 2]), (_quirk_P(), [0, 1, 2])]
    ops1 += [(g2[q], [q, q + 1]) for q in range(2, 6)]
    U1 = _fuse(ops1, list(range(0, 7)))

    ops2 = [(g1[q], [q]) for q in range(7, 14)]
    ops2 += [(g2[q], [q, q + 1]) for q in range(6, 13)]
    U2 = _fuse(ops2, list(range(6, 14)))  # q6 = MSB of the 256 index

    ops3 = [(g1[q], [q]) for q in range(14, 21)]
    ops3 += [(g2[q], [q, q + 1]) for q in range(13, 20)]
    U3 = _fuse(ops3, list(range(13, 21)))  # q13 = MSB

    ops4 = [(g1[q], [q]) for q in range(21, 24)]
    ops4 += [(g2[q], [q, q + 1]) for q in range(20, 23)]
    # partition index on the device = s*16 + m, s = (q21,q22,q23), m = (q17..q20)
    U4 = _fuse(ops4, [21, 22, 23, 17, 18, 19, 20])

    return U1, U2, U3, U4


def _pack_lhsT(U):
    """lhsT components for out = U @ x (complex):  A = re(U)^T, B = im(U)^T,
    Bn = -im(U)^T, stacked (3, n, n) float32."""
    return np.stack([U.real.T, U.imag.T, -U.imag.T]).astype(np.float32)


def build_weights(gates1, gates2):
    U1, U2, U3, U4 = build_chunk_matrices(gates1, gates2)
    w1 = _pack_lhsT(U1)
    w4 = _pack_lhsT(U4)

    def blocks(U):  # (2, 2, 3, 128, 128)
        return np.stack([
            np.stack([_pack_lhsT(U[j * 128:(j + 1) * 128, k * 128:(k + 1) * 128])
                      for k in (0, 1)])
            for j in (0, 1)])

    w2 = blocks(U2)
    w3 = blocks(U3)
    ident = np.eye(128, dtype=np.float32)
    return {"w1": w1, "w2": w2, "w3": w3, "w4": w4, "ident": ident}


# ---------------------------------------------------------------------------
# Bass kernel builder
# ---------------------------------------------------------------------------

def build_nc(use_f32r=USE_F32R):
    nc = bacc.Bacc()
    SD_DRAM = F32R if use_f32r else F32

    st = nc.declare_dram_parameter("state", [2, 1 << 21], F32, isOutput=False)
    w1 = nc.declare_dram_parameter("w1", [3, 128, 128], F32, isOutput=False)
    w2 = nc.declare_dram_parameter("w2", [2, 2, 3, 128, 128], F32, isOutput=False)
    w3 = nc.declare_dram_parameter("w3", [2, 2, 3, 128, 128], F32, isOutput=False)
    w4 = nc.declare_dram_parameter("w4", [3, 128, 128], F32, isOutput=False)
    idn = nc.declare_dram_parameter("ident", [128, 128], F32, isOutput=False)
    out = nc.declare_dram_parameter("out", [2, 1 << 21], F32, isOutput=True)

    # AllToAll bounce buffers, split in two halves by q3 so the collective
    # overlaps P3/P4 compute: [block(dest/src rank), plane, part, inner/2]
    a2a_in = [nc.dram_tensor(f"a2a_in{h}", [8, 2, 128, 1024], SD_DRAM)
              for h in (0, 1)]
    a2a_out = [nc.dram_tensor(f"a2a_out{h}", [8, 2, 128, 1024], SD_DRAM)
               for h in (0, 1)]

    SD = F32R if use_f32r else F32  # SBUF/bounce storage dtype

    def ldma(out_ap, in_ap):
        # DRAM f32 -> SBUF f32r needs the SWDGE cast path
        if use_f32r:
            nc.gpsimd.dma_start(out=out_ap, in_=in_ap)
        else:
            nc.sync.dma_start(out=out_ap, in_=in_ap)

    with tile.TileContext(nc, num_cores=N_CORES) as tc:
        with tc.tile_pool(name="state", bufs=1) as sp, \
             tc.tile_pool(name="wpool", bufs=1) as wp, \
             tc.tile_pool(name="mm", bufs=6, space="PSUM") as mmp, \
             tc.tile_pool(name="tr", bufs=2, space="PSUM") as trp:

            sre = sp.tile([128, 16384], SD, tag="sre")
            sim = sp.tile([128, 16384], SD, tag="sim")

            # ---- load weights ----
            def load_w3(dram_ap3, name):  # (3,128,128) -> 3 sbuf tiles
                ts = []
                for i in range(3):
                    t = wp.tile([128, 128], SD, tag=f"{name}_{i}")
                    ldma(t[:], dram_ap3[i])
                    ts.append(t)
                return ts

            w1t = load_w3(w1, "w1")
            w4t = load_w3(w4, "w4")
            w2t = [[load_w3(w2[j, k], f"w2_{j}{k}") for k in (0, 1)] for j in (0, 1)]
            w3t = [[load_w3(w3[j, k], f"w3_{j}{k}") for k in (0, 1)] for j in (0, 1)]
            idt = wp.tile([128, 128], SD, tag="ident")
            ldma(idt[:], idn[:])

            # ---- load state:  partitions (q0..q6), free (q7..q20) ----
            st_v = [st[pl].rearrange("(p f) -> p f", p=128) for pl in (0, 1)]
            for c in range(4):
                for pl, s in ((0, sre), (1, sim)):
                    ldma(s[:, c * 4096:(c + 1) * 4096],
                         st_v[pl][:, c * 4096:(c + 1) * 4096])

            planes = ((sre, sim))

            def cmul_into(pre, pim, W, xre, xim, start, stop=False):
                """pre += re(U)@xre - im(U)@xim ; pim += im(U)@xre + re(U)@xim
                W = [A, B, Bn] lhsT tiles."""
                A, B, Bn = W
                nc.tensor.matmul(pre[:], (A[:]), (xre), start=start, stop=False)
                nc.tensor.matmul(pim[:], (A[:]), (xim), start=start, stop=False)
                nc.tensor.matmul(pim[:], (B[:]), (xre), start=False, stop=stop)
                nc.tensor.matmul(pre[:], (Bn[:]), (xim), start=False, stop=stop)

            # ---- P1: chunk on partitions (q0..q6) ----
            for t in range(32):
                c0 = t * 512
                pre = mmp.tile([128, 512], F32, tag="mm")
                pim = mmp.tile([128, 512], F32, tag="mm")
                xre = sre[:, c0:c0 + 512]
                xim = sim[:, c0:c0 + 512]
                A, B, Bn = w1t
                nc.tensor.matmul(pre[:], (A[:]), (xre), start=True, stop=False)
                nc.tensor.matmul(pim[:], (A[:]), (xim), start=True, stop=False)
                nc.tensor.matmul(pim[:], (B[:]), (xre), start=False, stop=True)
                nc.tensor.matmul(pre[:], (Bn[:]), (xim), start=False, stop=True)
                nc.vector.tensor_copy(sre[:, c0:c0 + 512], pre[:])
                nc.scalar.copy(out=sim[:, c0:c0 + 512], in_=pim[:])

            # ---- T1: transpose partitions (q0..q6) <-> free (q7..q13) ----
            # L1 free = (q7..q13)*128 + (q14..q20); window w = (q14..q20):
            # read col-set {a*128+w}, transpose, write back to same col-set,
            # giving L2: partitions (q7..q13), free = (q0..q6)*128 + (q14..q20).
            for si, s in enumerate((sre, sim)):
                sv = s[:].rearrange("p (a w) -> p a w", w=128)
                for w in range(128):
                    pt = trp.tile([128, 128], SD, tag="tr")
                    nc.tensor.transpose((pt[:]), (sv[:, :, w]), (idt[:]))
                    if (w + si) % 3 == 0:
                        nc.scalar.copy(out=sv[:, :, w], in_=pt[:])
                    else:
                        nc.vector.tensor_copy(sv[:, :, w], pt[:])

            # ---- P2: chunk [6..13]; partitions (q7..q13), q6 = free bit ----
            # L2 free = (q0..q6)*128 + (q14..q20); q6 = bit0 of the outer
            # index => columns alternate 128-blocks by q6.
            sre_v = sre[:].rearrange("p (o q c) -> p o q c", q=2, c=128)
            sim_v = sim[:].rearrange("p (o q c) -> p o q c", q=2, c=128)
            for t in range(16):
                o0 = t * 4
                xr = [sre_v[:, o0:o0 + 4, k, :] for k in (0, 1)]
                xi = [sim_v[:, o0:o0 + 4, k, :] for k in (0, 1)]
                ps = []
                for j in (0, 1):
                    pre = mmp.tile([128, 512], F32, tag="mm")
                    pim = mmp.tile([128, 512], F32, tag="mm")
                    cmul_into(pre, pim, w2t[j][0], xr[0], xi[0], start=True)
                    cmul_into(pre, pim, w2t[j][1], xr[1], xi[1], start=False, stop=True)
                    ps.append((pre, pim))
                for j in (0, 1):
                    pre, pim = ps[j]
                    nc.vector.tensor_copy(sre_v[:, o0:o0 + 4, j, :], pre[:])
                    nc.scalar.copy(out=sim_v[:, o0:o0 + 4, j, :], in_=pim[:])

            # ---- T2: transpose partitions (q7..q13) <-> free (q14..q20) ----
            # window o = (q0..q6): read contiguous block [o*128, o*128+128),
            # write back contiguous, giving L3: partitions (q14..q20),
            # free = (q0..q6)*128 + (q7..q13) = (q0..q13) natural.
            for si, s in enumerate((sre, sim)):
                for o in range(128):
                    pt = trp.tile([128, 128], SD, tag="tr")
                    nc.tensor.transpose(
                        (pt[:]), (s[:, o * 128:o * 128 + 128]), (idt[:]))
                    if (o + si) % 3 == 0:
                        nc.scalar.copy(out=s[:, o * 128:o * 128 + 128], in_=pt[:])
                    else:
                        nc.vector.tensor_copy(s[:, o * 128:o * 128 + 128], pt[:])

            # ---- P3: chunk [13..20]; partitions (q14..q20), q13 = free bit0 ----
            sre_w = sre[:].rearrange("p (c k) -> p c k", k=2)
            sim_w = sim[:].rearrange("p (c k) -> p c k", k=2)

            def p3_tile(t):
                c0 = t * 512
                xr = [sre_w[:, c0:c0 + 512, k] for k in (0, 1)]
                xi = [sim_w[:, c0:c0 + 512, k] for k in (0, 1)]
                ps = []
                for j in (0, 1):
                    pre = mmp.tile([128, 512], F32, tag="mm")
                    pim = mmp.tile([128, 512], F32, tag="mm")
                    cmul_into(pre, pim, w3t[j][0], xr[0], xi[0], start=True)
                    cmul_into(pre, pim, w3t[j][1], xr[1], xi[1], start=False, stop=True)
                    ps.append((pre, pim))
                for j in (0, 1):
                    pre, pim = ps[j]
                    nc.vector.tensor_copy(sre_w[:, c0:c0 + 512, j], pre[:])
                    nc.scalar.copy(out=sim_w[:, c0:c0 + 512, j], in_=pim[:])

            def stage_half(h):
                # SBUF (part q14..q20, free (q0..q13)) ->
                # a2a_in[h][b=(q0q1q2), pl, part, (q4..q13)], q3 = h
                for pl, s in ((0, sre), (1, sim)):
                    sv = s[:].rearrange("p (b g f) -> p b g f", b=8, g=2)
                    nc.sync.dma_start(
                        out=a2a_in[h][:, pl].rearrange("b p f -> p b f"),
                        in_=sv[:, :, h, :])
                nc.gpsimd.collective_compute(
                    "AllToAll",
                    mybir.AluOpType.bypass,
                    replica_groups=[list(range(N_CORES))],
                    ins=[a2a_in[h].ap().opt()],
                    outs=[a2a_out[h].ap().opt()],
                )

            def readback_half(h):
                # a2a_out[h][s3, pl, (h3,m), f] -> partitions s3*16+m,
                # free = h3*2048 + h*1024 + f, f = (q4..q13)
                for s3 in range(8):
                    for pl, s in ((0, sre), (1, sim)):
                        sv = (s[s3 * 16:(s3 + 1) * 16, :]
                              .rearrange("m (h3 g f) -> m h3 g f", h3=8, g=2))
                        nc.sync.dma_start(
                            out=sv[:, :, h, :],
                            in_=a2a_out[h][s3, pl]
                                .rearrange("(h3 m) f -> m h3 f", m=16))

            for t in range(0, 16, 2):  # q3 = 0 tiles
                p3_tile(t)
            stage_half(0)
            for t in range(1, 16, 2):  # q3 = 1 tiles
                p3_tile(t)
            stage_half(1)
            readback_half(0)
            readback_half(1)

            # ---- P4: chunk [20..23] on partitions (q21,q22,q23,q17..q20) ----
            p4_order = [t for t in range(32) if (t >> 1) & 1 == 0] + \
                       [t for t in range(32) if (t >> 1) & 1 == 1]
            for t in p4_order:
                c0 = t * 512
                pre = mmp.tile([128, 512], F32, tag="mm")
                pim = mmp.tile([128, 512], F32, tag="mm")
                xre = sre[:, c0:c0 + 512]
                xim = sim[:, c0:c0 + 512]
                A, B, Bn = w4t
                nc.tensor.matmul(pre[:], (A[:]), (xre), start=True, stop=False)
                nc.tensor.matmul(pim[:], (A[:]), (xim), start=True, stop=False)
                nc.tensor.matmul(pim[:], (B[:]), (xre), start=False, stop=True)
                nc.tensor.matmul(pre[:], (Bn[:]), (xim), start=False, stop=True)
                nc.vector.tensor_copy(sre[:, c0:c0 + 512], pre[:])
                nc.scalar.copy(out=sim[:, c0:c0 + 512], in_=pim[:])
                # store this 512-col chunk as soon as it's final
                for pl, s in ((0, sre), (1, sim)):
                    ov = out[pl].rearrange("(p f) -> p f", p=128)
                    nc.sync.dma_start(
                        out=ov[:, c0:c0 + 512],
                        in_=(s[:, c0:c0 + 512].bitcast(F32)
                             if use_f32r else s[:, c0:c0 + 512]))

    return nc


# ---------------------------------------------------------------------------
# Host wrapper
# ---------------------------------------------------------------------------

TRACE = False          # set by test harnesses to capture a profile
LAST_EXEC_NS = None
LAST_RESULTS = None


def kernel(state, gates1, gates2):
    global LAST_EXEC_NS, LAST_RESULTS
    state = np.ascontiguousarray(np.asarray(state, dtype=np.float32))
    weights = build_weights(np.asarray(gates1, dtype=np.float32),
                            np.asarray(gates2, dtype=np.float32))

    # shard over (q21,q22,q23) = index mod 8
    shards = np.ascontiguousarray(
        state.reshape(2, 1 << 21, 8).transpose(2, 0, 1))

    nc = build_nc()
    if not nc.is_finalized():
        nc.finalize()
    in_maps = [dict(weights, state=shards[d]) for d in range(N_CORES)]
    res = run_bass_kernel_spmd(nc, in_maps, core_ids=list(range(N_CORES)),
                               trace=TRACE)
    LAST_EXEC_NS = res.exec_time_ns
    LAST_RESULTS = res

    # unshard: core d holds (q0,q1,q2) = d;
    # out layout = [plane][s=(q21..q23), m=(q17..q20)][h=(q14..q16)][f=(q3..q13)]
    full = np.empty((2, 8, 2048, 8, 16, 8), dtype=np.float32)
    for d in range(N_CORES):
        od = res.results[d]["out"].reshape(2, 8, 16, 8, 2048)
        full[:, d] = od.transpose(0, 4, 3, 2, 1)
    return full.reshape(2, 1 << 24)


if __name__ == "__main__":
    rng = np.random.default_rng(0)
    state = rng.standard_normal((2, 1 << 24)).astype(np.float32)
    g1 = rng.standard_normal((24, 2, 2, 2)).astype(np.float32)
    g2 = rng.standard_normal((23, 2, 4, 4)).astype(np.float32)
    out = kernel(state, g1, g2)
    print(out.shape, out.dtype)


# revision 15
# speedup vs baseline: 1.0725x; 1.0725x over previous
"""Trainium2 Bass kernel for nn_Circuit_19275813225041.

24-qubit state-vector simulation: one layer of single-qubit gates on every
qubit, then a ladder of two-qubit gates on neighboring pairs (q, q+1),
q = 0..22, on a 2^24 complex state stored as (2, 2^24) float32 (re, im).

Strategy (8 NeuronCores):
  - Qubit q <-> bit q of the state index, bit 0 = MSB.
  - Shard the state over the 3 LSB qubits (q21,q22,q23): core d holds
    amplitudes with index % 8 == d (state-vector slicing).
  - Gates are fused on the host into 4 big chunk matrices:
      U1: 128x128 on qubits [0..6]    (singles 0..6, ladder (0,1)..(5,6))
      U2: 256x256 on qubits [6..13]   (singles 7..13, ladder (6,7)..(12,13))
      U3: 256x256 on qubits [13..20]  (singles 14..20, ladder (13,14)..(19,20))
      U4: 128x128 on qubits [21,22,23,17..20] (singles 21..23, ladder
          (20,21),(21,22),(22,23), identity on q17..q19)
    (The reference's _apply_gate has a permutation quirk for the 2-qubit
    gate at q=1 -- its "inverse" transpose applies perm again, which for
    q=1 is a 3-cycle.  This adds a relabeling permutation on qubits
    (0,1,2) right after that gate; it is folded into U1.)
  - Each core applies U1..U3 to its local 2^21 state via TensorE matmuls,
    with PE transposes rotating 7-bit groups through the partition axis and
    2-term PSUM accumulation handling the chunk boundary bit (q6, q13).
  - One AllToAll swaps qubits (q0,q1,q2) <-> (q21,q22,q23) across cores so
    the final chunk U4 applies locally; output is returned sharded over
    (q0,q1,q2) and reassembled on the host.
"""

import numpy as np

import concourse.bass as bass
import concourse.bacc as bacc
import concourse.mybir as mybir
import concourse.tile as tile
from concourse.bass_utils import run_bass_kernel_spmd

F32 = mybir.dt.float32
F32R = mybir.dt.float32r

USE_F32R = True   # fast fp32 matmul mode (reduced precision)
N_CORES = 8


# ---------------------------------------------------------------------------
# Host-side gate fusion
# ---------------------------------------------------------------------------

def _embed_gate(mat, qubits, group):
    """Embed `mat` acting on `qubits` (MSB-first) into the space indexed by
    `group` (list of qubits, group[0] = MSB of the index)."""
    g = len(group)
    k = len(qubits)
    pos = [group.index(q) for q in qubits]
    rest = [i for i in range(g) if i not in pos]
    U = np.zeros((1 << g, 1 << g), dtype=np.complex128)
    for r in range(1 << len(rest)):
        base = 0
        for bi, p in enumerate(rest):
            if (r >> (len(rest) - 1 - bi)) & 1:
                base |= 1 << (g - 1 - p)
        for a in range(1 << k):
            ia = base
            for bi, p in enumerate(pos):
                if (a >> (k - 1 - bi)) & 1:
                    ia |= 1 << (g - 1 - p)
            for b in range(1 << k):
                ib = base
                for bi, p in enumerate(pos):
                    if (b >> (k - 1 - bi)) & 1:
                        ib |= 1 << (g - 1 - p)
                U[ia, ib] = mat[a, b]
    return U


def _quirk_P():
    # reference._apply_gate on [1,2]: the un-permute uses perm (a 3-cycle)
    # instead of its inverse => extra relabeling on qubits (0,1,2):
    # new (b0,b1,b2) = (old b2, old b0, old b1).
    P = np.zeros((8, 8), dtype=np.complex128)
    for b0 in range(2):
        for b1 in range(2):
            for b2 in range(2):
                P[(b2 << 2) | (b0 << 1) | b1, (b0 << 2) | (b1 << 1) | b2] = 1
    return P


def _fuse(ops, group):
    U = np.eye(1 << len(group), dtype=np.complex128)
    for mat, qb in ops:
        U = _embed_gate(mat, qb, group) @ U
    return U


def build_chunk_matrices(gates1, gates2):
    g1 = gates1[:, 0].astype(np.float64) + 1j * gates1[:, 1].astype(np.float64)
    g2 = gates2[:, 0].astype(np.float64) + 1j * gates2[:, 1].astype(np.float64)

    ops1 = [(g1[q], [q]) for q in range(0, 7)]
    ops1 += [(g2[0], [0, 1]), (g2[1], [1, 2]), (_quirk_P(), [0, 1, 2])]
    ops1 += [(g2[q], [q, q + 1]) for q in range(2, 6)]
    U1 = _fuse(ops1, list(range(0, 7)))

    ops2 = [(g1[q], [q]) for q in range(7, 14)]
    ops2 += [(g2[q], [q, q + 1]) for q in range(6, 13)]
    U2 = _fuse(ops2, list(range(6, 14)))  # q6 = MSB of the 256 index

    ops3 = [(g1[q], [q]) for q in range(14, 21)]
    ops3 += [(g2[q], [q, q + 1]) for q in range(13, 20)]
    U3 = _fuse(ops3, list(range(13, 21)))  # q13 = MSB

    ops4 = [(g1[q], [q]) for q in range(21, 24)]
    ops4 += [(g2[q], [q, q + 1]) for q in range(20, 23)]
    # partition index on the device = s*16 + m, s = (q21,q22,q23), m = (q17..q20)
    U4 = _fuse(ops4, [21, 22, 23, 17, 18, 19, 20])

    return U1, U2, U3, U4


def _pack_lhsT(U):
    """lhsT components for out = U @ x (complex):  A = re(U)^T, B = im(U)^T,
    Bn = -im(U)^T, stacked (3, n, n) float32."""
    return np.stack([U.real.T, U.imag.T, -U.imag.T]).astype(np.float32)


def build_weights(gates1, gates2):
    U1, U2, U3, U4 = build_chunk_matrices(gates1, gates2)
    w1 = _pack_lhsT(U1)
    w4 = _pack_lhsT(U4)

    def blocks(U):  # (2, 2, 3, 128, 128)
        return np.stack([
            np.stack([_pack_lhsT(U[j * 128:(j + 1) * 128, k * 128:(k + 1) * 128])
                      for k in (0, 1)])
            for j in (0, 1)])

    w2 = blocks(U2)
    w3 = blocks(U3)
    ident = np.eye(128, dtype=np.float32)
    return {"w1": w1, "w2": w2, "w3": w3, "w4": w4, "ident": ident}


# ---------------------------------------------------------------------------
# Bass kernel builder
# ---------------------------------------------------------------------------

def build_nc(use_f32r=USE_F32R):
    nc = bacc.Bacc()
    SD_DRAM = F32R if use_f32r else F32

    st = nc.declare_dram_parameter("state", [2, 1 << 21], F32, isOutput=False)
    w1 = nc.declare_dram_parameter("w1", [3, 128, 128], F32, isOutput=False)
    w2 = nc.declare_dram_parameter("w2", [2, 2, 3, 128, 128], F32, isOutput=False)
    w3 = nc.declare_dram_parameter("w3", [2, 2, 3, 128, 128], F32, isOutput=False)
    w4 = nc.declare_dram_parameter("w4", [3, 128, 128], F32, isOutput=False)
    idn = nc.declare_dram_parameter("ident", [128, 128], F32, isOutput=False)
    out = nc.declare_dram_parameter("out", [2, 1 << 21], F32, isOutput=True)

    # AllToAll bounce buffers, split in two halves by q3 so the collective
    # overlaps P3/P4 compute: [block(dest/src rank), plane, part, inner/2]
    a2a_in = [nc.dram_tensor(f"a2a_in{h}", [8, 2, 128, 1024], SD_DRAM)
              for h in (0, 1)]
    a2a_out = [nc.dram_tensor(f"a2a_out{h}", [8, 2, 128, 1024], SD_DRAM)
               for h in (0, 1)]

    SD = F32R if use_f32r else F32  # SBUF/bounce storage dtype

    def ldma(out_ap, in_ap):
        # DRAM f32 -> SBUF f32r needs the SWDGE cast path
        if use_f32r:
            nc.gpsimd.dma_start(out=out_ap, in_=in_ap)
        else:
            nc.sync.dma_start(out=out_ap, in_=in_ap)

    with tile.TileContext(nc, num_cores=N_CORES) as tc:
        with tc.tile_pool(name="state", bufs=1) as sp, \
             tc.tile_pool(name="wpool", bufs=1) as wp, \
             tc.tile_pool(name="mm", bufs=6, space="PSUM") as mmp, \
             tc.tile_pool(name="tr", bufs=2, space="PSUM") as trp:

            sre = sp.tile([128, 16384], SD, tag="sre")
            sim = sp.tile([128, 16384], SD, tag="sim")

            # ---- load weights ----
            def load_w3(dram_ap3, name):  # (3,128,128) -> 3 sbuf tiles
                ts = []
                for i in range(3):
                    t = wp.tile([128, 128], SD, tag=f"{name}_{i}")
                    ldma(t[:], dram_ap3[i])
                    ts.append(t)
                return ts

            w1t = load_w3(w1, "w1")
            w4t = load_w3(w4, "w4")
            w2t = [[load_w3(w2[j, k], f"w2_{j}{k}") for k in (0, 1)] for j in (0, 1)]
            w3t = [[load_w3(w3[j, k], f"w3_{j}{k}") for k in (0, 1)] for j in (0, 1)]
            idt = wp.tile([128, 128], SD, tag="ident")
            ldma(idt[:], idn[:])

            # ---- load state:  partitions (q0..q6), free (q7..q20) ----
            st_v = [st[pl].rearrange("(p f) -> p f", p=128) for pl in (0, 1)]
            for c in range(4):
                for pl, s in ((0, sre), (1, sim)):
                    ldma(s[:, c * 4096:(c + 1) * 4096],
                         st_v[pl][:, c * 4096:(c + 1) * 4096])

            planes = ((sre, sim))

            def cmul_into(pre, pim, W, xre, xim, start, stop=False):
                """pre += re(U)@xre - im(U)@xim ; pim += im(U)@xre + re(U)@xim
                W = [A, B, Bn] lhsT tiles."""
                A, B, Bn = W
                nc.tensor.matmul(pre[:], (A[:]), (xre), start=start, stop=False)
                nc.tensor.matmul(pim[:], (A[:]), (xim), start=start, stop=False)
                nc.tensor.matmul(pim[:], (B[:]), (xre), start=False, stop=stop)
                nc.tensor.matmul(pre[:], (Bn[:]), (xim), start=False, stop=stop)

            # ---- P1: chunk on partitions (q0..q6) ----
            for t in range(32):
                c0 = t * 512
                pre = mmp.tile([128, 512], F32, tag="mm")
                pim = mmp.tile([128, 512], F32, tag="mm")
                xre = sre[:, c0:c0 + 512]
                xim = sim[:, c0:c0 + 512]
                A, B, Bn = w1t
                nc.tensor.matmul(pre[:], (A[:]), (xre), start=True, stop=False)
                nc.tensor.matmul(pim[:], (A[:]), (xim), start=True, stop=False)
                nc.tensor.matmul(pim[:], (B[:]), (xre), start=False, stop=True)
                nc.tensor.matmul(pre[:], (Bn[:]), (xim), start=False, stop=True)
                nc.vector.tensor_copy(sre[:, c0:c0 + 512], pre[:])
                nc.scalar.copy(out=sim[:, c0:c0 + 512], in_=pim[:])

            # ---- T1: transpose partitions (q0..q6) <-> free (q7..q13) ----
            # L1 free = (q7..q13)*128 + (q14..q20); window w = (q14..q20):
            # read col-set {a*128+w}, transpose, write back to same col-set,
            # giving L2: partitions (q7..q13), free = (q0..q6)*128 + (q14..q20).
            for si, s in enumerate((sre, sim)):
                sv = s[:].rearrange("p (a w) -> p a w", w=128)
                for w in range(128):
                    pt = trp.tile([128, 128], SD, tag="tr")
                    nc.tensor.transpose((pt[:]), (sv[:, :, w]), (idt[:]))
                    if (w + si) % 3 == 0:
                        nc.scalar.copy(out=sv[:, :, w], in_=pt[:])
                    else:
                        nc.vector.tensor_copy(sv[:, :, w], pt[:])

            # ---- P2: chunk [6..13]; partitions (q7..q13), q6 = free bit ----
            # L2 free = (q0..q6)*128 + (q14..q20); q6 = bit0 of the outer
            # index => columns alternate 128-blocks by q6.
            sre_v = sre[:].rearrange("p (o q c) -> p o q c", q=2, c=128)
            sim_v = sim[:].rearrange("p (o q c) -> p o q c", q=2, c=128)
            for t in range(16):
                o0 = t * 4
                xr = [sre_v[:, o0:o0 + 4, k, :] for k in (0, 1)]
                xi = [sim_v[:, o0:o0 + 4, k, :] for k in (0, 1)]
                ps = []
                for j in (0, 1):
                    pre = mmp.tile([128, 512], F32, tag="mm")
                    pim = mmp.tile([128, 512], F32, tag="mm")
                    cmul_into(pre, pim, w2t[j][0], xr[0], xi[0], start=True)
                    cmul_into(pre, pim, w2t[j][1], xr[1], xi[1], start=False, stop=True)
                    ps.append((pre, pim))
                for j in (0, 1):
                    pre, pim = ps[j]
                    nc.vector.tensor_copy(sre_v[:, o0:o0 + 4, j, :], pre[:])
                    nc.scalar.copy(out=sim_v[:, o0:o0 + 4, j, :], in_=pim[:])

            # ---- T2: transpose partitions (q7..q13) <-> free (q14..q20) ----
            # window o = (q0..q6): read contiguous block [o*128, o*128+128),
            # write back contiguous, giving L3: partitions (q14..q20),
            # free = (q0..q6)*128 + (q7..q13) = (q0..q13) natural.
            # PSUM col j = (q7..q12)*2 + q13; write back with q13 at
            # stride 64 so P3's k13-halves are contiguous 64-elem runs.
            for si, s in enumerate((sre, sim)):
                sv2 = s[:].rearrange("p (o k c) -> p o k c", k=2, c=64)
                for o in range(128):
                    pt = trp.tile([128, 128], SD, tag="tr")
                    nc.tensor.transpose(
                        (pt[:]), (s[:, o * 128:o * 128 + 128]), (idt[:]))
                    ptv = pt[:].rearrange("p (c k) -> p k c", k=2)
                    if (o + si) % 3 == 0:
                        nc.scalar.copy(out=sv2[:, o], in_=ptv)
                    else:
                        nc.vector.tensor_copy(sv2[:, o], ptv)

            # ---- P3: chunk [13..20]; partitions (q14..q20), q13 = free bit0 ----
            sre_w = sre[:].rearrange("p (o k b) -> p o k b", k=2, b=64)
            sim_w = sim[:].rearrange("p (o k b) -> p o k b", k=2, b=64)

            def p3_tile(t):
                o0 = t * 8
                xr = [sre_w[:, o0:o0 + 8, k, :] for k in (0, 1)]
                xi = [sim_w[:, o0:o0 + 8, k, :] for k in (0, 1)]
                ps = []
                for j in (0, 1):
                    pre = mmp.tile([128, 512], F32, tag="mm")
                    pim = mmp.tile([128, 512], F32, tag="mm")
                    cmul_into(pre, pim, w3t[j][0], xr[0], xi[0], start=True)
                    cmul_into(pre, pim, w3t[j][1], xr[1], xi[1], start=False, stop=True)
                    ps.append((pre, pim))
                for j in (0, 1):
                    pre, pim = ps[j]
                    nc.vector.tensor_copy(sre_w[:, o0:o0 + 8, j, :], pre[:])
                    nc.scalar.copy(out=sim_w[:, o0:o0 + 8, j, :], in_=pim[:])

            def stage_half(h):
                # SBUF (part q14..q20, free (q0..q13)) ->
                # a2a_in[h][b=(q0q1q2), pl, part, (q4..q13)], q3 = h
                for pl, s in ((0, sre), (1, sim)):
                    sv = s[:].rearrange("p (b g f) -> p b g f", b=8, g=2)
                    nc.sync.dma_start(
                        out=a2a_in[h][:, pl].rearrange("b p f -> p b f"),
                        in_=sv[:, :, h, :])
                nc.gpsimd.collective_compute(
                    "AllToAll",
                    mybir.AluOpType.bypass,
                    replica_groups=[list(range(N_CORES))],
                    ins=[a2a_in[h].ap().opt()],
                    outs=[a2a_out[h].ap().opt()],
                )

            def readback_half(h):
                # a2a_out[h][s3, pl, (h3,m), f] -> partitions s3*16+m,
                # free = h3*2048 + h*1024 + f, f = (q4..q13)
                for s3 in range(8):
                    for pl, s in ((0, sre), (1, sim)):
                        sv = (s[s3 * 16:(s3 + 1) * 16, :]
                              .rearrange("m (h3 g f) -> m h3 g f", h3=8, g=2))
                        nc.sync.dma_start(
                            out=sv[:, :, h, :],
                            in_=a2a_out[h][s3, pl]
                                .rearrange("(h3 m) f -> m h3 f", m=16))

            for t in range(0, 16, 2):  # q3 = 0 tiles
                p3_tile(t)
            stage_half(0)
            for t in range(1, 16, 2):  # q3 = 1 tiles
                p3_tile(t)
            stage_half(1)
            readback_half(0)
            readback_half(1)

            # ---- P4: chunk [20..23] on partitions (q21,q22,q23,q17..q20) ----
            p4_order = [t for t in range(32) if (t >> 1) & 1 == 0] + \
                       [t for t in range(32) if (t >> 1) & 1 == 1]
            for t in p4_order:
                c0 = t * 512
                pre = mmp.tile([128, 512], F32, tag="mm")
                pim = mmp.tile([128, 512], F32, tag="mm")
                xre = sre[:, c0:c0 + 512]
                xim = sim[:, c0:c0 + 512]
                A, B, Bn = w4t
                nc.tensor.matmul(pre[:], (A[:]), (xre), start=True, stop=False)
                nc.tensor.matmul(pim[:], (A[:]), (xim), start=True, stop=False)
                nc.tensor.matmul(pim[:], (B[:]), (xre), start=False, stop=True)
                nc.tensor.matmul(pre[:], (Bn[:]), (xim), start=False, stop=True)
                nc.vector.tensor_copy(sre[:, c0:c0 + 512], pre[:])
                nc.scalar.copy(out=sim[:, c0:c0 + 512], in_=pim[:])
                # store this 512-col chunk as soon as it's final
                for pl, s in ((0, sre), (1, sim)):
                    ov = out[pl].rearrange("(p f) -> p f", p=128)
                    nc.sync.dma_start(
                        out=ov[:, c0:c0 + 512],
                        in_=(s[:, c0:c0 + 512].bitcast(F32)
                             if use_f32r else s[:, c0:c0 + 512]))

    return nc


# ---------------------------------------------------------------------------
# Host wrapper
# ---------------------------------------------------------------------------

TRACE = False          # set by test harnesses to capture a profile
LAST_EXEC_NS = None
LAST_RESULTS = None


def kernel(state, gates1, gates2):
    global LAST_EXEC_NS, LAST_RESULTS
    state = np.ascontiguousarray(np.asarray(state, dtype=np.float32))
    weights = build_weights(np.asarray(gates1, dtype=np.float32),
                            np.asarray(gates2, dtype=np.float32))

    # shard over (q21,q22,q23) = index mod 8
    shards = np.ascontiguousarray(
        state.reshape(2, 1 << 21, 8).transpose(2, 0, 1))

    nc = build_nc()
    if not nc.is_finalized():
        nc.finalize()
    in_maps = [dict(weights, state=shards[d]) for d in range(N_CORES)]
    res = run_bass_kernel_spmd(nc, in_maps, core_ids=list(range(N_CORES)),
                               trace=TRACE)
    LAST_EXEC_NS = res.exec_time_ns
    LAST_RESULTS = res

    return unshard([res.results[d]["out"] for d in range(N_CORES)])


def unshard(outs):
    # core d holds (q0,q1,q2) = d;
    # out layout = [plane][s=(q21..q23), m=(q17..q20)][h3=(q14..q16)]
    #              [a=(q3..q6)][k=(q13)][c=(q7..q12)]
    full = np.empty((2, 8, 16, 64, 2, 8, 16, 8), dtype=np.float32)
    for d in range(N_CORES):
        od = np.asarray(outs[d]).reshape(2, 8, 16, 8, 16, 2, 64)
        full[:, d] = od.transpose(0, 4, 6, 5, 3, 2, 1)
    return full.reshape(2, 1 << 24)


if __name__ == "__main__":
    rng = np.random.default_rng(0)
    state = rng.standard_normal((2, 1 << 24)).astype(np.float32)
    g1 = rng.standard_normal((24, 2, 2, 2)).astype(np.float32)
    g2 = rng.standard_normal((23, 2, 4, 4)).astype(np.float32)
    out = kernel(state, g1, g2)
    print(out.shape, out.dtype)


# revision 18
# speedup vs baseline: 1.1150x; 1.0396x over previous
"""Trainium2 Bass kernel for nn_Circuit_19275813225041.

24-qubit state-vector simulation: one layer of single-qubit gates on every
qubit, then a ladder of two-qubit gates on neighboring pairs (q, q+1),
q = 0..22, on a 2^24 complex state stored as (2, 2^24) float32 (re, im).

Strategy (8 NeuronCores):
  - Qubit q <-> bit q of the state index, bit 0 = MSB.
  - Shard the state over the 3 LSB qubits (q21,q22,q23): core d holds
    amplitudes with index % 8 == d (state-vector slicing).
  - Gates are fused on the host into 4 big chunk matrices:
      U1: 128x128 on qubits [0..6]    (singles 0..6, ladder (0,1)..(5,6))
      U2: 256x256 on qubits [6..13]   (singles 7..13, ladder (6,7)..(12,13))
      U3: 256x256 on qubits [13..20]  (singles 14..20, ladder (13,14)..(19,20))
      U4: 128x128 on qubits [21,22,23,17..20] (singles 21..23, ladder
          (20,21),(21,22),(22,23), identity on q17..q19)
    (The reference's _apply_gate has a permutation quirk for the 2-qubit
    gate at q=1 -- its "inverse" transpose applies perm again, which for
    q=1 is a 3-cycle.  This adds a relabeling permutation on qubits
    (0,1,2) right after that gate; it is folded into U1.)
  - Each core applies U1..U3 to its local 2^21 state via TensorE matmuls,
    with PE transposes rotating 7-bit groups through the partition axis and
    2-term PSUM accumulation handling the chunk boundary bit (q6, q13).
  - One AllToAll swaps qubits (q0,q1,q2) <-> (q21,q22,q23) across cores so
    the final chunk U4 applies locally; output is returned sharded over
    (q0,q1,q2) and reassembled on the host.
"""

import numpy as np

import concourse.bass as bass
import concourse.bacc as bacc
import concourse.mybir as mybir
import concourse.tile as tile
from concourse.bass_utils import run_bass_kernel_spmd

F32 = mybir.dt.float32
F32R = mybir.dt.float32r

USE_F32R = True   # fast fp32 matmul mode (reduced precision)
N_CORES = 8


# ---------------------------------------------------------------------------
# Host-side gate fusion
# ---------------------------------------------------------------------------

def _embed_gate(mat, qubits, group):
    """Embed `mat` acting on `qubits` (MSB-first) into the space indexed by
    `group` (list of qubits, group[0] = MSB of the index)."""
    g = len(group)
    k = len(qubits)
    pos = [group.index(q) for q in qubits]
    rest = [i for i in range(g) if i not in pos]
    U = np.zeros((1 << g, 1 << g), dtype=np.complex128)
    for r in range(1 << len(rest)):
        base = 0
        for bi, p in enumerate(rest):
            if (r >> (len(rest) - 1 - bi)) & 1:
                base |= 1 << (g - 1 - p)
        for a in range(1 << k):
            ia = base
            for bi, p in enumerate(pos):
                if (a >> (k - 1 - bi)) & 1:
                    ia |= 1 << (g - 1 - p)
            for b in range(1 << k):
                ib = base
                for bi, p in enumerate(pos):
                    if (b >> (k - 1 - bi)) & 1:
                        ib |= 1 << (g - 1 - p)
                U[ia, ib] = mat[a, b]
    return U


def _quirk_P():
    # reference._apply_gate on [1,2]: the un-permute uses perm (a 3-cycle)
    # instead of its inverse => extra relabeling on qubits (0,1,2):
    # new (b0,b1,b2) = (old b2, old b0, old b1).
    P = np.zeros((8, 8), dtype=np.complex128)
    for b0 in range(2):
        for b1 in range(2):
            for b2 in range(2):
                P[(b2 << 2) | (b0 << 1) | b1, (b0 << 2) | (b1 << 1) | b2] = 1
    return P


def _fuse(ops, group):
    U = np.eye(1 << len(group), dtype=np.complex128)
    for mat, qb in ops:
        U = _embed_gate(mat, qb, group) @ U
    return U


def build_chunk_matrices(gates1, gates2):
    g1 = gates1[:, 0].astype(np.float64) + 1j * gates1[:, 1].astype(np.float64)
    g2 = gates2[:, 0].astype(np.float64) + 1j * gates2[:, 1].astype(np.float64)

    ops1 = [(g1[q], [q]) for q in range(0, 7)]
    ops1 += [(g2[0], [0, 1]), (g2[1], [1, 2]), (_quirk_P(), [0, 1, 2])]
    ops1 += [(g2[q], [q, q + 1]) for q in range(2, 6)]
    U1 = _fuse(ops1, list(range(0, 7)))

    ops2 = [(g1[q], [q]) for q in range(7, 14)]
    ops2 += [(g2[q], [q, q + 1]) for q in range(6, 13)]
    U2 = _fuse(ops2, list(range(6, 14)))  # q6 = MSB of the 256 index

    ops3 = [(g1[q], [q]) for q in range(14, 21)]
    ops3 += [(g2[q], [q, q + 1]) for q in range(13, 20)]
    U3 = _fuse(ops3, list(range(13, 21)))  # q13 = MSB

    ops4 = [(g1[q], [q]) for q in range(21, 24)]
    ops4 += [(g2[q], [q, q + 1]) for q in range(20, 23)]
    # partition index on the device = s*16 + m, s = (q21,q22,q23), m = (q17..q20)
    U4 = _fuse(ops4, [21, 22, 23, 17, 18, 19, 20])

    return U1, U2, U3, U4


def _pack_lhsT(U):
    """lhsT components for out = U @ x (complex):  A = re(U)^T, B = im(U)^T,
    Bn = -im(U)^T, stacked (3, n, n) float32."""
    return np.stack([U.real.T, U.imag.T, -U.imag.T]).astype(np.float32)


def build_weights(gates1, gates2):
    U1, U2, U3, U4 = build_chunk_matrices(gates1, gates2)
    w1 = _pack_lhsT(U1)
    w4 = _pack_lhsT(U4)

    def blocks(U):  # (2, 2, 3, 128, 128)
        return np.stack([
            np.stack([_pack_lhsT(U[j * 128:(j + 1) * 128, k * 128:(k + 1) * 128])
                      for k in (0, 1)])
            for j in (0, 1)])

    w2 = blocks(U2)
    w3 = blocks(U3)
    ident = np.eye(128, dtype=np.float32)
    return {"w1": w1, "w2": w2, "w3": w3, "w4": w4, "ident": ident}


# ---------------------------------------------------------------------------
# Bass kernel builder
# ---------------------------------------------------------------------------

def build_nc(use_f32r=USE_F32R):
    nc = bacc.Bacc()
    SD_DRAM = F32R if use_f32r else F32

    st = nc.declare_dram_parameter("state", [2, 1 << 21], F32, isOutput=False)
    w1 = nc.declare_dram_parameter("w1", [3, 128, 128], F32, isOutput=False)
    w2 = nc.declare_dram_parameter("w2", [2, 2, 3, 128, 128], F32, isOutput=False)
    w3 = nc.declare_dram_parameter("w3", [2, 2, 3, 128, 128], F32, isOutput=False)
    w4 = nc.declare_dram_parameter("w4", [3, 128, 128], F32, isOutput=False)
    idn = nc.declare_dram_parameter("ident", [128, 128], F32, isOutput=False)
    out = nc.declare_dram_parameter("out", [2, 1 << 21], F32, isOutput=True)

    # AllToAll bounce buffers, split in four quarters by (q3,q4) so the
    # collective overlaps P3/P4 compute: [block(rank), plane, part, inner/4]
    a2a_in = [nc.dram_tensor(f"a2a_in{h}", [8, 2, 128, 512], SD_DRAM)
              for h in range(4)]
    a2a_out = [nc.dram_tensor(f"a2a_out{h}", [8, 2, 128, 512], SD_DRAM)
               for h in range(4)]

    SD = F32R if use_f32r else F32  # SBUF/bounce storage dtype

    def ldma(out_ap, in_ap):
        # DRAM f32 -> SBUF f32r needs the SWDGE cast path
        if use_f32r:
            nc.gpsimd.dma_start(out=out_ap, in_=in_ap)
        else:
            nc.sync.dma_start(out=out_ap, in_=in_ap)

    with tile.TileContext(nc, num_cores=N_CORES) as tc:
        with tc.tile_pool(name="state", bufs=1) as sp, \
             tc.tile_pool(name="wpool", bufs=1) as wp, \
             tc.tile_pool(name="mm", bufs=6, space="PSUM") as mmp, \
             tc.tile_pool(name="tr", bufs=2, space="PSUM") as trp:

            sre = sp.tile([128, 16384], SD, tag="sre")
            sim = sp.tile([128, 16384], SD, tag="sim")

            # ---- load weights ----
            def load_w3(dram_ap3, name):  # (3,128,128) -> 3 sbuf tiles
                ts = []
                for i in range(3):
                    t = wp.tile([128, 128], SD, tag=f"{name}_{i}")
                    ldma(t[:], dram_ap3[i])
                    ts.append(t)
                return ts

            w1t = load_w3(w1, "w1")
            w4t = load_w3(w4, "w4")
            w2t = [[load_w3(w2[j, k], f"w2_{j}{k}") for k in (0, 1)] for j in (0, 1)]
            w3t = [[load_w3(w3[j, k], f"w3_{j}{k}") for k in (0, 1)] for j in (0, 1)]
            idt = wp.tile([128, 128], SD, tag="ident")
            ldma(idt[:], idn[:])

            # ---- load state:  partitions (q0..q6), free (q7..q20) ----
            st_v = [st[pl].rearrange("(p f) -> p f", p=128) for pl in (0, 1)]
            for c in range(4):
                for pl, s in ((0, sre), (1, sim)):
                    ldma(s[:, c * 4096:(c + 1) * 4096],
                         st_v[pl][:, c * 4096:(c + 1) * 4096])

            planes = ((sre, sim))

            def cmul_into(pre, pim, W, xre, xim, start, stop=False):
                """pre += re(U)@xre - im(U)@xim ; pim += im(U)@xre + re(U)@xim
                W = [A, B, Bn] lhsT tiles."""
                A, B, Bn = W
                nc.tensor.matmul(pre[:], (A[:]), (xre), start=start, stop=False)
                nc.tensor.matmul(pim[:], (A[:]), (xim), start=start, stop=False)
                nc.tensor.matmul(pim[:], (B[:]), (xre), start=False, stop=stop)
                nc.tensor.matmul(pre[:], (Bn[:]), (xim), start=False, stop=stop)

            # ---- P1: chunk on partitions (q0..q6) ----
            # tiles processed in pairs, matmuls grouped by weight so the
            # stationary operand reloads 3x per 8 matmuls instead of 8x
            def simple_pass_pair(W, t, u):
                A, B, Bn = W
                ps = []
                xs = []
                for tt in (t, u):
                    c0 = tt * 512
                    pre = mmp.tile([128, 512], F32, tag="mm")
                    pim = mmp.tile([128, 512], F32, tag="mm")
                    ps.append((pre, pim))
                    xs.append((sre[:, c0:c0 + 512], sim[:, c0:c0 + 512]))
                for i in (0, 1):
                    nc.tensor.matmul(ps[i][0][:], (A[:]), xs[i][0], start=True, stop=False)
                    nc.tensor.matmul(ps[i][1][:], (A[:]), xs[i][1], start=True, stop=False)
                for i in (0, 1):
                    nc.tensor.matmul(ps[i][1][:], (B[:]), xs[i][0], start=False, stop=True)
                for i in (0, 1):
                    nc.tensor.matmul(ps[i][0][:], (Bn[:]), xs[i][1], start=False, stop=True)
                for i, tt in enumerate((t, u)):
                    c0 = tt * 512
                    nc.vector.tensor_copy(sre[:, c0:c0 + 512], ps[i][0][:])
                    nc.scalar.copy(out=sim[:, c0:c0 + 512], in_=ps[i][1][:])

            for t in range(0, 32, 2):
                simple_pass_pair(w1t, t, t + 1)

            # ---- T1: transpose partitions (q0..q6) <-> free (q7..q13) ----
            # L1 free = (q7..q13)*128 + (q14..q20); window w = (q14..q20):
            # read col-set {a*128+w}, transpose, write back to same col-set,
            # giving L2: partitions (q7..q13), free = (q0..q6)*128 + (q14..q20).
            for si, s in enumerate((sre, sim)):
                sv = s[:].rearrange("p (a w) -> p a w", w=128)
                for w in range(128):
                    pt = trp.tile([128, 128], SD, tag="tr")
                    nc.tensor.transpose((pt[:]), (sv[:, :, w]), (idt[:]))
                    if (w + si) % 3 == 0:
                        nc.scalar.copy(out=sv[:, :, w], in_=pt[:])
                    else:
                        nc.vector.tensor_copy(sv[:, :, w], pt[:])

            # ---- P2: chunk [6..13]; partitions (q7..q13), q6 = free bit ----
            # L2 free = (q0..q6)*128 + (q14..q20); q6 = bit0 of the outer
            # index => columns alternate 128-blocks by q6.
            sre_v = sre[:].rearrange("p (o q c) -> p o q c", q=2, c=128)
            sim_v = sim[:].rearrange("p (o q c) -> p o q c", q=2, c=128)
            for t in range(16):
                o0 = t * 4
                xr = [sre_v[:, o0:o0 + 4, k, :] for k in (0, 1)]
                xi = [sim_v[:, o0:o0 + 4, k, :] for k in (0, 1)]
                ps = []
                for j in (0, 1):
                    pre = mmp.tile([128, 512], F32, tag="mm")
                    pim = mmp.tile([128, 512], F32, tag="mm")
                    cmul_into(pre, pim, w2t[j][0], xr[0], xi[0], start=True)
                    cmul_into(pre, pim, w2t[j][1], xr[1], xi[1], start=False, stop=True)
                    ps.append((pre, pim))
                for j in (0, 1):
                    pre, pim = ps[j]
                    nc.vector.tensor_copy(sre_v[:, o0:o0 + 4, j, :], pre[:])
                    nc.scalar.copy(out=sim_v[:, o0:o0 + 4, j, :], in_=pim[:])

            # ---- T2: transpose partitions (q7..q13) <-> free (q14..q20) ----
            # window o = (q0..q6): read contiguous block [o*128, o*128+128),
            # write back contiguous, giving L3: partitions (q14..q20),
            # free = (q0..q6)*128 + (q7..q13) = (q0..q13) natural.
            # PSUM col j = (q7..q12)*2 + q13; write back with q13 at
            # stride 64 so P3's k13-halves are contiguous 64-elem runs.
            for si, s in enumerate((sre, sim)):
                sv2 = s[:].rearrange("p (o k c) -> p o k c", k=2, c=64)
                for o in range(128):
                    pt = trp.tile([128, 128], SD, tag="tr")
                    nc.tensor.transpose(
                        (pt[:]), (s[:, o * 128:o * 128 + 128]), (idt[:]))
                    ptv = pt[:].rearrange("p (c k) -> p k c", k=2)
                    if (o + si) % 3 == 0:
                        nc.scalar.copy(out=sv2[:, o], in_=ptv)
                    else:
                        nc.vector.tensor_copy(sv2[:, o], ptv)

            # ---- P3: chunk [13..20]; partitions (q14..q20), q13 = free bit0 ----
            sre_w = sre[:].rearrange("p (o k b) -> p o k b", k=2, b=64)
            sim_w = sim[:].rearrange("p (o k b) -> p o k b", k=2, b=64)

            def p3_tile(t):
                o0 = t * 4
                xr = [sre_w[:, o0:o0 + 4, k, :] for k in (0, 1)]
                xi = [sim_w[:, o0:o0 + 4, k, :] for k in (0, 1)]
                ps = []
                for j in (0, 1):
                    pre = mmp.tile([128, 256], F32, tag="mm")
                    pim = mmp.tile([128, 256], F32, tag="mm")
                    cmul_into(pre, pim, w3t[j][0], xr[0], xi[0], start=True)
                    cmul_into(pre, pim, w3t[j][1], xr[1], xi[1], start=False, stop=True)
                    ps.append((pre, pim))
                for j in (0, 1):
                    pre, pim = ps[j]
                    nc.vector.tensor_copy(sre_w[:, o0:o0 + 4, j, :], pre[:])
                    nc.scalar.copy(out=sim_w[:, o0:o0 + 4, j, :], in_=pim[:])

            def stage_quarter(v):
                # SBUF (part q14..q20, free (q0..q13)) ->
                # a2a_in[v][b=(q0q1q2), pl, part, f], (q3,q4) = v
                for pl, s in ((0, sre), (1, sim)):
                    sv = s[:].rearrange("p (b g f) -> p b g f", b=8, g=4)
                    nc.sync.dma_start(
                        out=a2a_in[v][:, pl].rearrange("b p f -> p b f"),
                        in_=sv[:, :, v, :])
                nc.gpsimd.collective_compute(
                    "AllToAll",
                    mybir.AluOpType.bypass,
                    replica_groups=[list(range(N_CORES))],
                    ins=[a2a_in[v].ap().opt()],
                    outs=[a2a_out[v].ap().opt()],
                )

            def readback_quarter(v):
                # a2a_out[v][s3, pl, (h3,m), f] -> partitions s3*16+m,
                # free = h3*2048 + v*512 + f
                for s3 in range(8):
                    for pl, s in ((0, sre), (1, sim)):
                        sv = (s[s3 * 16:(s3 + 1) * 16, :]
                              .rearrange("m (h3 g f) -> m h3 g f", h3=8, g=4))
                        nc.sync.dma_start(
                            out=sv[:, :, v, :],
                            in_=a2a_out[v][s3, pl]
                                .rearrange("(h3 m) f -> m h3 f", m=16))

            # P3 tile t = (q0..q4); quarter v = (q3,q4) = t & 3
            for v in range(4):
                for t in range(32):
                    if t & 3 == v:
                        p3_tile(t)
                stage_quarter(v)
            for v in range(4):
                readback_quarter(v)

            # ---- P4: chunk [20..23] on partitions (q21,q22,q23,q17..q20) ----
            # P4 tile t covers free [512t, 512t+512) = fixes
            # (q14q15q16, q3, q4); order by A2A quarter v = (q3,q4)
            p4_order = [t for v in range(4) for t in range(32) if (t & 3) == v]
            for i in range(0, 32, 2):
                t, u = p4_order[i], p4_order[i + 1]
                simple_pass_pair(w4t, t, u)
                for tt in (t, u):
                    c0 = tt * 512
                    for pl, s in ((0, sre), (1, sim)):
                        ov = out[pl].rearrange("(p f) -> p f", p=128)
                        nc.sync.dma_start(
                            out=ov[:, c0:c0 + 512],
                            in_=(s[:, c0:c0 + 512].bitcast(F32)
                                 if use_f32r else s[:, c0:c0 + 512]))

    return nc


# ---------------------------------------------------------------------------
# Host wrapper
# ---------------------------------------------------------------------------

TRACE = False          # set by test harnesses to capture a profile
LAST_EXEC_NS = None
LAST_RESULTS = None


def kernel(state, gates1, gates2):
    global LAST_EXEC_NS, LAST_RESULTS
    state = np.ascontiguousarray(np.asarray(state, dtype=np.float32))
    weights = build_weights(np.asarray(gates1, dtype=np.float32),
                            np.asarray(gates2, dtype=np.float32))

    # shard over (q21,q22,q23) = index mod 8
    shards = np.ascontiguousarray(
        state.reshape(2, 1 << 21, 8).transpose(2, 0, 1))

    nc = build_nc()
    if not nc.is_finalized():
        nc.finalize()
    in_maps = [dict(weights, state=shards[d]) for d in range(N_CORES)]
    res = run_bass_kernel_spmd(nc, in_maps, core_ids=list(range(N_CORES)),
                               trace=TRACE)
    LAST_EXEC_NS = res.exec_time_ns
    LAST_RESULTS = res

    return unshard([res.results[d]["out"] for d in range(N_CORES)])


def unshard(outs):
    # core d holds (q0,q1,q2) = d;
    # out layout = [plane][s=(q21..q23), m=(q17..q20)][h3=(q14..q16)]
    #              [a=(q3..q6)][k=(q13)][c=(q7..q12)]
    full = np.empty((2, 8, 16, 64, 2, 8, 16, 8), dtype=np.float32)
    for d in range(N_CORES):
        od = np.asarray(outs[d]).reshape(2, 8, 16, 8, 16, 2, 64)
        full[:, d] = od.transpose(0, 4, 6, 5, 3, 2, 1)
    return full.reshape(2, 1 << 24)


if __name__ == "__main__":
    rng = np.random.default_rng(0)
    state = rng.standard_normal((2, 1 << 24)).astype(np.float32)
    g1 = rng.standard_normal((24, 2, 2, 2)).astype(np.float32)
    g2 = rng.standard_normal((23, 2, 4, 4)).astype(np.float32)
    out = kernel(state, g1, g2)
    print(out.shape, out.dtype)


# revision 19
# speedup vs baseline: 1.2707x; 1.1397x over previous
"""Trainium2 Bass kernel for nn_Circuit_19275813225041.

24-qubit state-vector simulation: one layer of single-qubit gates on every
qubit, then a ladder of two-qubit gates on neighboring pairs (q, q+1),
q = 0..22, on a 2^24 complex state stored as (2, 2^24) float32 (re, im).

Strategy (8 NeuronCores):
  - Qubit q <-> bit q of the state index, bit 0 = MSB.
  - Shard the state over the 3 LSB qubits (q21,q22,q23): core d holds
    amplitudes with index % 8 == d (state-vector slicing).
  - Gates are fused on the host into 4 big chunk matrices:
      U1: 128x128 on qubits [0..6]    (singles 0..6, ladder (0,1)..(5,6))
      U2: 256x256 on qubits [6..13]   (singles 7..13, ladder (6,7)..(12,13))
      U3: 256x256 on qubits [13..20]  (singles 14..20, ladder (13,14)..(19,20))
      U4: 128x128 on qubits [21,22,23,17..20] (singles 21..23, ladder
          (20,21),(21,22),(22,23), identity on q17..q19)
    (The reference's _apply_gate has a permutation quirk for the 2-qubit
    gate at q=1 -- its "inverse" transpose applies perm again, which for
    q=1 is a 3-cycle.  This adds a relabeling permutation on qubits
    (0,1,2) right after that gate; it is folded into U1.)
  - Each core applies U1..U3 to its local 2^21 state via TensorE matmuls,
    with PE transposes rotating 7-bit groups through the partition axis and
    2-term PSUM accumulation handling the chunk boundary bit (q6, q13).
  - One AllToAll swaps qubits (q0,q1,q2) <-> (q21,q22,q23) across cores so
    the final chunk U4 applies locally; output is returned sharded over
    (q0,q1,q2) and reassembled on the host.
"""

import numpy as np

import concourse.bass as bass
import concourse.bacc as bacc
import concourse.mybir as mybir
import concourse.tile as tile
from concourse.bass_utils import run_bass_kernel_spmd

F32 = mybir.dt.float32
F32R = mybir.dt.float32r
BF16 = mybir.dt.bfloat16

USE_F32R = True   # fast fp32 matmul mode (reduced precision)
N_CORES = 8


# ---------------------------------------------------------------------------
# Host-side gate fusion
# ---------------------------------------------------------------------------

def _embed_gate(mat, qubits, group):
    """Embed `mat` acting on `qubits` (MSB-first) into the space indexed by
    `group` (list of qubits, group[0] = MSB of the index)."""
    g = len(group)
    k = len(qubits)
    pos = [group.index(q) for q in qubits]
    rest = [i for i in range(g) if i not in pos]
    U = np.zeros((1 << g, 1 << g), dtype=np.complex128)
    for r in range(1 << len(rest)):
        base = 0
        for bi, p in enumerate(rest):
            if (r >> (len(rest) - 1 - bi)) & 1:
                base |= 1 << (g - 1 - p)
        for a in range(1 << k):
            ia = base
            for bi, p in enumerate(pos):
                if (a >> (k - 1 - bi)) & 1:
                    ia |= 1 << (g - 1 - p)
            for b in range(1 << k):
                ib = base
                for bi, p in enumerate(pos):
                    if (b >> (k - 1 - bi)) & 1:
                        ib |= 1 << (g - 1 - p)
                U[ia, ib] = mat[a, b]
    return U


def _quirk_P():
    # reference._apply_gate on [1,2]: the un-permute uses perm (a 3-cycle)
    # instead of its inverse => extra relabeling on qubits (0,1,2):
    # new (b0,b1,b2) = (old b2, old b0, old b1).
    P = np.zeros((8, 8), dtype=np.complex128)
    for b0 in range(2):
        for b1 in range(2):
            for b2 in range(2):
                P[(b2 << 2) | (b0 << 1) | b1, (b0 << 2) | (b1 << 1) | b2] = 1
    return P


def _fuse(ops, group):
    U = np.eye(1 << len(group), dtype=np.complex128)
    for mat, qb in ops:
        U = _embed_gate(mat, qb, group) @ U
    return U


def build_chunk_matrices(gates1, gates2):
    g1 = gates1[:, 0].astype(np.float64) + 1j * gates1[:, 1].astype(np.float64)
    g2 = gates2[:, 0].astype(np.float64) + 1j * gates2[:, 1].astype(np.float64)

    ops1 = [(g1[q], [q]) for q in range(0, 7)]
    ops1 += [(g2[0], [0, 1]), (g2[1], [1, 2]), (_quirk_P(), [0, 1, 2])]
    ops1 += [(g2[q], [q, q + 1]) for q in range(2, 6)]
    U1 = _fuse(ops1, list(range(0, 7)))

    ops2 = [(g1[q], [q]) for q in range(7, 14)]
    ops2 += [(g2[q], [q, q + 1]) for q in range(6, 13)]
    U2 = _fuse(ops2, list(range(6, 14)))  # q6 = MSB of the 256 index

    ops3 = [(g1[q], [q]) for q in range(14, 21)]
    ops3 += [(g2[q], [q, q + 1]) for q in range(13, 20)]
    U3 = _fuse(ops3, list(range(13, 21)))  # q13 = MSB

    ops4 = [(g1[q], [q]) for q in range(21, 24)]
    ops4 += [(g2[q], [q, q + 1]) for q in range(20, 23)]
    # partition index on the device = s*16 + m, s = (q21,q22,q23), m = (q17..q20)
    U4 = _fuse(ops4, [21, 22, 23, 17, 18, 19, 20])

    return U1, U2, U3, U4


def _pack_lhsT(U):
    """lhsT components for out = U @ x (complex):  A = re(U)^T, B = im(U)^T,
    Bn = -im(U)^T, stacked (3, n, n) float32."""
    return np.stack([U.real.T, U.imag.T, -U.imag.T]).astype(np.float32)


def build_weights(gates1, gates2):
    U1, U2, U3, U4 = build_chunk_matrices(gates1, gates2)
    w1 = _pack_lhsT(U1)
    w4 = _pack_lhsT(U4)

    def blocks(U):  # (2, 2, 3, 128, 128)
        return np.stack([
            np.stack([_pack_lhsT(U[j * 128:(j + 1) * 128, k * 128:(k + 1) * 128])
                      for k in (0, 1)])
            for j in (0, 1)])

    w2 = blocks(U2)
    w3 = blocks(U3)
    ident = np.eye(128, dtype=np.float32)
    return {"w1": w1, "w2": w2, "w3": w3, "w4": w4, "ident": ident}


# ---------------------------------------------------------------------------
# Bass kernel builder
# ---------------------------------------------------------------------------

def build_nc(use_f32r=USE_F32R):
    nc = bacc.Bacc()
    SD_DRAM = F32R if use_f32r else F32

    st = nc.declare_dram_parameter("state", [2, 1 << 21], F32, isOutput=False)
    w1 = nc.declare_dram_parameter("w1", [3, 128, 128], F32, isOutput=False)
    w2 = nc.declare_dram_parameter("w2", [2, 2, 3, 128, 128], F32, isOutput=False)
    w3 = nc.declare_dram_parameter("w3", [2, 2, 3, 128, 128], F32, isOutput=False)
    w4 = nc.declare_dram_parameter("w4", [3, 128, 128], F32, isOutput=False)
    idn = nc.declare_dram_parameter("ident", [128, 128], F32, isOutput=False)
    out = nc.declare_dram_parameter("out", [2, 1 << 21], F32, isOutput=True)

    # AllToAll bounce buffers, split in four quarters by (q3,q4) so the
    # collective overlaps P3/P4 compute. bf16 wire format halves the bytes
    # (precision budget: f32r matmuls already give ~3e-4; bf16 transport of
    # the state adds ~4e-3, well under the 2e-2 gate).
    a2a_in = [nc.dram_tensor(f"a2a_in{h}", [8, 2, 128, 512], BF16)
              for h in range(4)]
    a2a_out = [nc.dram_tensor(f"a2a_out{h}", [8, 2, 128, 512], BF16)
               for h in range(4)]

    SD = F32R if use_f32r else F32  # SBUF/bounce storage dtype

    def ldma(out_ap, in_ap):
        # DRAM f32 -> SBUF f32r needs the SWDGE cast path
        if use_f32r:
            nc.gpsimd.dma_start(out=out_ap, in_=in_ap)
        else:
            nc.sync.dma_start(out=out_ap, in_=in_ap)

    with tile.TileContext(nc, num_cores=N_CORES) as tc:
        with tc.tile_pool(name="state", bufs=1) as sp, \
             tc.tile_pool(name="wpool", bufs=1) as wp, \
             tc.tile_pool(name="mm", bufs=6, space="PSUM") as mmp, \
             tc.tile_pool(name="tr", bufs=2, space="PSUM") as trp:

            sre = sp.tile([128, 16384], SD, tag="sre")
            sim = sp.tile([128, 16384], SD, tag="sim")
            # bf16 staging for the A2A wire (2 rotating quarter-slots)
            stg_re = sp.tile([128, 8192], BF16, tag="stg_re")
            stg_im = sp.tile([128, 8192], BF16, tag="stg_im")

            # ---- load weights ----
            def load_w3(dram_ap3, name):  # (3,128,128) -> 3 sbuf tiles
                ts = []
                for i in range(3):
                    t = wp.tile([128, 128], SD, tag=f"{name}_{i}")
                    ldma(t[:], dram_ap3[i])
                    ts.append(t)
                return ts

            w1t = load_w3(w1, "w1")
            w4t = load_w3(w4, "w4")
            w2t = [[load_w3(w2[j, k], f"w2_{j}{k}") for k in (0, 1)] for j in (0, 1)]
            w3t = [[load_w3(w3[j, k], f"w3_{j}{k}") for k in (0, 1)] for j in (0, 1)]
            idt = wp.tile([128, 128], SD, tag="ident")
            ldma(idt[:], idn[:])

            # ---- load state:  partitions (q0..q6), free (q7..q20) ----
            st_v = [st[pl].rearrange("(p f) -> p f", p=128) for pl in (0, 1)]
            for c in range(4):
                for pl, s in ((0, sre), (1, sim)):
                    ldma(s[:, c * 4096:(c + 1) * 4096],
                         st_v[pl][:, c * 4096:(c + 1) * 4096])

            planes = ((sre, sim))

            def cmul_into(pre, pim, W, xre, xim, start, stop=False):
                """pre += re(U)@xre - im(U)@xim ; pim += im(U)@xre + re(U)@xim
                W = [A, B, Bn] lhsT tiles."""
                A, B, Bn = W
                nc.tensor.matmul(pre[:], (A[:]), (xre), start=start, stop=False)
                nc.tensor.matmul(pim[:], (A[:]), (xim), start=start, stop=False)
                nc.tensor.matmul(pim[:], (B[:]), (xre), start=False, stop=stop)
                nc.tensor.matmul(pre[:], (Bn[:]), (xim), start=False, stop=stop)

            # ---- P1: chunk on partitions (q0..q6) ----
            # tiles processed in pairs, matmuls grouped by weight so the
            # stationary operand reloads 3x per 8 matmuls instead of 8x
            def simple_pass_pair(W, t, u):
                A, B, Bn = W
                ps = []
                xs = []
                for tt in (t, u):
                    c0 = tt * 512
                    pre = mmp.tile([128, 512], F32, tag="mm")
                    pim = mmp.tile([128, 512], F32, tag="mm")
                    ps.append((pre, pim))
                    xs.append((sre[:, c0:c0 + 512], sim[:, c0:c0 + 512]))
                for i in (0, 1):
                    nc.tensor.matmul(ps[i][0][:], (A[:]), xs[i][0], start=True, stop=False)
                    nc.tensor.matmul(ps[i][1][:], (A[:]), xs[i][1], start=True, stop=False)
                for i in (0, 1):
                    nc.tensor.matmul(ps[i][1][:], (B[:]), xs[i][0], start=False, stop=True)
                for i in (0, 1):
                    nc.tensor.matmul(ps[i][0][:], (Bn[:]), xs[i][1], start=False, stop=True)
                for i, tt in enumerate((t, u)):
                    c0 = tt * 512
                    nc.vector.tensor_copy(sre[:, c0:c0 + 512], ps[i][0][:])
                    nc.scalar.copy(out=sim[:, c0:c0 + 512], in_=ps[i][1][:])

            for t in range(0, 32, 2):
                simple_pass_pair(w1t, t, t + 1)

            # ---- T1: transpose partitions (q0..q6) <-> free (q7..q13) ----
            # L1 free = (q7..q13)*128 + (q14..q20); window w = (q14..q20):
            # read col-set {a*128+w}, transpose, write back to same col-set,
            # giving L2: partitions (q7..q13), free = (q0..q6)*128 + (q14..q20).
            for si, s in enumerate((sre, sim)):
                sv = s[:].rearrange("p (a w) -> p a w", w=128)
                for w in range(128):
                    pt = trp.tile([128, 128], SD, tag="tr")
                    nc.tensor.transpose((pt[:]), (sv[:, :, w]), (idt[:]))
                    if (w + si) % 3 == 0:
                        nc.scalar.copy(out=sv[:, :, w], in_=pt[:])
                    else:
                        nc.vector.tensor_copy(sv[:, :, w], pt[:])

            # ---- P2: chunk [6..13]; partitions (q7..q13), q6 = free bit ----
            # L2 free = (q0..q6)*128 + (q14..q20); q6 = bit0 of the outer
            # index => columns alternate 128-blocks by q6.
            sre_v = sre[:].rearrange("p (o q c) -> p o q c", q=2, c=128)
            sim_v = sim[:].rearrange("p (o q c) -> p o q c", q=2, c=128)
            for t in range(16):
                o0 = t * 4
                xr = [sre_v[:, o0:o0 + 4, k, :] for k in (0, 1)]
                xi = [sim_v[:, o0:o0 + 4, k, :] for k in (0, 1)]
                ps = []
                for j in (0, 1):
                    pre = mmp.tile([128, 512], F32, tag="mm")
                    pim = mmp.tile([128, 512], F32, tag="mm")
                    cmul_into(pre, pim, w2t[j][0], xr[0], xi[0], start=True)
                    cmul_into(pre, pim, w2t[j][1], xr[1], xi[1], start=False, stop=True)
                    ps.append((pre, pim))
                for j in (0, 1):
                    pre, pim = ps[j]
                    nc.vector.tensor_copy(sre_v[:, o0:o0 + 4, j, :], pre[:])
                    nc.scalar.copy(out=sim_v[:, o0:o0 + 4, j, :], in_=pim[:])

            # ---- T2: transpose partitions (q7..q13) <-> free (q14..q20) ----
            # window o = (q0..q6): read contiguous block [o*128, o*128+128),
            # write back contiguous, giving L3: partitions (q14..q20),
            # free = (q0..q6)*128 + (q7..q13) = (q0..q13) natural.
            # PSUM col j = (q7..q12)*2 + q13; write back with q13 at
            # stride 64 so P3's k13-halves are contiguous 64-elem runs.
            for si, s in enumerate((sre, sim)):
                sv2 = s[:].rearrange("p (o k c) -> p o k c", k=2, c=64)
                for o in range(128):
                    pt = trp.tile([128, 128], SD, tag="tr")
                    nc.tensor.transpose(
                        (pt[:]), (s[:, o * 128:o * 128 + 128]), (idt[:]))
                    ptv = pt[:].rearrange("p (c k) -> p k c", k=2)
                    if (o + si) % 3 == 0:
                        nc.scalar.copy(out=sv2[:, o], in_=ptv)
                    else:
                        nc.vector.tensor_copy(sv2[:, o], ptv)

            # ---- P3: chunk [13..20]; partitions (q14..q20), q13 = free bit0 ----
            sre_w = sre[:].rearrange("p (o k b) -> p o k b", k=2, b=64)
            sim_w = sim[:].rearrange("p (o k b) -> p o k b", k=2, b=64)

            # staging views: slot = quarter v % 2, each slot 4096 cols
            stg_re_w = stg_re[:].rearrange("p (s o k b) -> p s o k b",
                                           s=2, k=2, b=64)
            stg_im_w = stg_im[:].rearrange("p (s o k b) -> p s o k b",
                                           s=2, k=2, b=64)

            def p3_tile(t):
                o0 = t * 4
                slot = (t & 3) % 2
                so = (t >> 2) * 4  # position of this tile inside its slot
                xr = [sre_w[:, o0:o0 + 4, k, :] for k in (0, 1)]
                xi = [sim_w[:, o0:o0 + 4, k, :] for k in (0, 1)]
                ps = []
                for j in (0, 1):
                    pre = mmp.tile([128, 256], F32, tag="mm")
                    pim = mmp.tile([128, 256], F32, tag="mm")
                    cmul_into(pre, pim, w3t[j][0], xr[0], xi[0], start=True)
                    cmul_into(pre, pim, w3t[j][1], xr[1], xi[1], start=False, stop=True)
                    ps.append((pre, pim))
                for j in (0, 1):
                    pre, pim = ps[j]
                    nc.vector.tensor_copy(stg_re_w[:, slot, so:so + 4, j, :], pre[:])
                    nc.scalar.copy(out=stg_im_w[:, slot, so:so + 4, j, :], in_=pim[:])

            def stage_quarter(v):
                # staging slot (part q14..q20, [b=(q0q1q2), f]) -> a2a_in[v]
                slot = v % 2
                for pl, s in ((0, stg_re), (1, stg_im)):
                    sv = s[:].rearrange("p (t b f) -> p t b f", t=2, b=8)
                    nc.sync.dma_start(
                        out=a2a_in[v][:, pl].rearrange("b p f -> p b f"),
                        in_=sv[:, slot, :, :])
                nc.gpsimd.collective_compute(
                    "AllToAll",
                    mybir.AluOpType.bypass,
                    replica_groups=[list(range(N_CORES))],
                    ins=[a2a_in[v].ap().opt()],
                    outs=[a2a_out[v].ap().opt()],
                )

            def readback_quarter(v):
                # a2a_out[v][s3, pl, (h3,m), f] -> partitions s3*16+m,
                # free = h3*2048 + v*512 + f   (bf16 -> f32r cast via SWDGE)
                for s3 in range(8):
                    for pl, s in ((0, sre), (1, sim)):
                        sv = (s[s3 * 16:(s3 + 1) * 16, :]
                              .rearrange("m (h3 g f) -> m h3 g f", h3=8, g=4))
                        nc.gpsimd.dma_start(
                            out=sv[:, :, v, :],
                            in_=a2a_out[v][s3, pl]
                                .rearrange("(h3 m) f -> m h3 f", m=16))

            # P3 tile t = (q0..q4); quarter v = (q3,q4) = t & 3
            for v in range(4):
                for t in range(32):
                    if t & 3 == v:
                        p3_tile(t)
                stage_quarter(v)


            # ---- P4: chunk [20..23] on partitions (q21,q22,q23,q17..q20) ----
            # P4 tile t covers free [512t, 512t+512) = fixes
            # (q14q15q16, q3, q4); quarter v = t & 3
            for v in range(4):
                readback_quarter(v)
                tiles = [t for t in range(32) if (t & 3) == v]
                for i in range(0, 8, 2):
                    t, u = tiles[i], tiles[i + 1]
                    simple_pass_pair(w4t, t, u)
                    for tt in (t, u):
                        c0 = tt * 512
                        for pl, s in ((0, sre), (1, sim)):
                            ov = out[pl].rearrange("(p f) -> p f", p=128)
                            nc.sync.dma_start(
                                out=ov[:, c0:c0 + 512],
                                in_=(s[:, c0:c0 + 512].bitcast(F32)
                                     if use_f32r else s[:, c0:c0 + 512]))

    return nc


# ---------------------------------------------------------------------------
# Host wrapper
# ---------------------------------------------------------------------------

TRACE = False          # set by test harnesses to capture a profile
LAST_EXEC_NS = None
LAST_RESULTS = None


def kernel(state, gates1, gates2):
    global LAST_EXEC_NS, LAST_RESULTS
    state = np.ascontiguousarray(np.asarray(state, dtype=np.float32))
    weights = build_weights(np.asarray(gates1, dtype=np.float32),
                            np.asarray(gates2, dtype=np.float32))

    # shard over (q21,q22,q23) = index mod 8
    shards = np.ascontiguousarray(
        state.reshape(2, 1 << 21, 8).transpose(2, 0, 1))

    nc = build_nc()
    if not nc.is_finalized():
        nc.finalize()
    in_maps = [dict(weights, state=shards[d]) for d in range(N_CORES)]
    res = run_bass_kernel_spmd(nc, in_maps, core_ids=list(range(N_CORES)),
                               trace=TRACE)
    LAST_EXEC_NS = res.exec_time_ns
    LAST_RESULTS = res

    return unshard([res.results[d]["out"] for d in range(N_CORES)])


def unshard(outs):
    # core d holds (q0,q1,q2) = d;
    # out layout = [plane][s=(q21..q23), m=(q17..q20)][h3=(q14..q16)]
    #              [a=(q3..q6)][k=(q13)][c=(q7..q12)]
    full = np.empty((2, 8, 16, 64, 2, 8, 16, 8), dtype=np.float32)
    for d in range(N_CORES):
        od = np.asarray(outs[d]).reshape(2, 8, 16, 8, 16, 2, 64)
        full[:, d] = od.transpose(0, 4, 6, 5, 3, 2, 1)
    return full.reshape(2, 1 << 24)


if __name__ == "__main__":
    rng = np.random.default_rng(0)
    state = rng.standard_normal((2, 1 << 24)).astype(np.float32)
    g1 = rng.standard_normal((24, 2, 2, 2)).astype(np.float32)
    g2 = rng.standard_normal((23, 2, 4, 4)).astype(np.float32)
    out = kernel(state, g1, g2)
    print(out.shape, out.dtype)


# revision 20
# speedup vs baseline: 1.3700x; 1.0781x over previous
"""Trainium2 Bass kernel for nn_Circuit_19275813225041.

24-qubit state-vector simulation: one layer of single-qubit gates on every
qubit, then a ladder of two-qubit gates on neighboring pairs (q, q+1),
q = 0..22, on a 2^24 complex state stored as (2, 2^24) float32 (re, im).

Strategy (8 NeuronCores):
  - Qubit q <-> bit q of the state index, bit 0 = MSB.
  - Shard the state over the 3 LSB qubits (q21,q22,q23): core d holds
    amplitudes with index % 8 == d (state-vector slicing).
  - Gates are fused on the host into 4 big chunk matrices:
      U1: 128x128 on qubits [0..6]    (singles 0..6, ladder (0,1)..(5,6))
      U2: 256x256 on qubits [6..13]   (singles 7..13, ladder (6,7)..(12,13))
      U3: 256x256 on qubits [13..20]  (singles 14..20, ladder (13,14)..(19,20))
      U4: 128x128 on qubits [21,22,23,17..20] (singles 21..23, ladder
          (20,21),(21,22),(22,23), identity on q17..q19)
    (The reference's _apply_gate has a permutation quirk for the 2-qubit
    gate at q=1 -- its "inverse" transpose applies perm again, which for
    q=1 is a 3-cycle.  This adds a relabeling permutation on qubits
    (0,1,2) right after that gate; it is folded into U1.)
  - Each core applies U1..U3 to its local 2^21 state via TensorE matmuls,
    with PE transposes rotating 7-bit groups through the partition axis and
    2-term PSUM accumulation handling the chunk boundary bit (q6, q13).
  - One AllToAll swaps qubits (q0,q1,q2) <-> (q21,q22,q23) across cores so
    the final chunk U4 applies locally; output is returned sharded over
    (q0,q1,q2) and reassembled on the host.
"""

import numpy as np

import concourse.bass as bass
import concourse.bacc as bacc
import concourse.mybir as mybir
import concourse.tile as tile
from concourse.bass_utils import run_bass_kernel_spmd

F32 = mybir.dt.float32
F32R = mybir.dt.float32r
BF16 = mybir.dt.bfloat16

USE_F32R = True   # fast fp32 matmul mode (reduced precision)
N_CORES = 8


# ---------------------------------------------------------------------------
# Host-side gate fusion
# ---------------------------------------------------------------------------

def _embed_gate(mat, qubits, group):
    """Embed `mat` acting on `qubits` (MSB-first) into the space indexed by
    `group` (list of qubits, group[0] = MSB of the index)."""
    g = len(group)
    k = len(qubits)
    pos = [group.index(q) for q in qubits]
    rest = [i for i in range(g) if i not in pos]
    U = np.zeros((1 << g, 1 << g), dtype=np.complex128)
    for r in range(1 << len(rest)):
        base = 0
        for bi, p in enumerate(rest):
            if (r >> (len(rest) - 1 - bi)) & 1:
                base |= 1 << (g - 1 - p)
        for a in range(1 << k):
            ia = base
            for bi, p in enumerate(pos):
                if (a >> (k - 1 - bi)) & 1:
                    ia |= 1 << (g - 1 - p)
            for b in range(1 << k):
                ib = base
                for bi, p in enumerate(pos):
                    if (b >> (k - 1 - bi)) & 1:
                        ib |= 1 << (g - 1 - p)
                U[ia, ib] = mat[a, b]
    return U


def _quirk_P():
    # reference._apply_gate on [1,2]: the un-permute uses perm (a 3-cycle)
    # instead of its inverse => extra relabeling on qubits (0,1,2):
    # new (b0,b1,b2) = (old b2, old b0, old b1).
    P = np.zeros((8, 8), dtype=np.complex128)
    for b0 in range(2):
        for b1 in range(2):
            for b2 in range(2):
                P[(b2 << 2) | (b0 << 1) | b1, (b0 << 2) | (b1 << 1) | b2] = 1
    return P


def _fuse(ops, group):
    U = np.eye(1 << len(group), dtype=np.complex128)
    for mat, qb in ops:
        U = _embed_gate(mat, qb, group) @ U
    return U


def build_chunk_matrices(gates1, gates2):
    g1 = gates1[:, 0].astype(np.float64) + 1j * gates1[:, 1].astype(np.float64)
    g2 = gates2[:, 0].astype(np.float64) + 1j * gates2[:, 1].astype(np.float64)

    ops1 = [(g1[q], [q]) for q in range(0, 7)]
    ops1 += [(g2[0], [0, 1]), (g2[1], [1, 2]), (_quirk_P(), [0, 1, 2])]
    ops1 += [(g2[q], [q, q + 1]) for q in range(2, 6)]
    U1 = _fuse(ops1, list(range(0, 7)))

    ops2 = [(g1[q], [q]) for q in range(7, 14)]
    ops2 += [(g2[q], [q, q + 1]) for q in range(6, 13)]
    U2 = _fuse(ops2, list(range(6, 14)))  # q6 = MSB of the 256 index

    ops3 = [(g1[q], [q]) for q in range(14, 21)]
    ops3 += [(g2[q], [q, q + 1]) for q in range(13, 20)]
    U3 = _fuse(ops3, list(range(13, 21)))  # q13 = MSB

    ops4 = [(g1[q], [q]) for q in range(21, 24)]
    ops4 += [(g2[q], [q, q + 1]) for q in range(20, 23)]
    # partition index on the device = s*16 + m, s = (q21,q22,q23), m = (q17..q20)
    U4 = _fuse(ops4, [21, 22, 23, 17, 18, 19, 20])

    return U1, U2, U3, U4


def _pack_lhsT(U):
    """lhsT components for out = U @ x (complex):  A = re(U)^T, B = im(U)^T,
    Bn = -im(U)^T, stacked (3, n, n) float32."""
    return np.stack([U.real.T, U.imag.T, -U.imag.T]).astype(np.float32)


def build_weights(gates1, gates2):
    U1, U2, U3, U4 = build_chunk_matrices(gates1, gates2)
    w1 = _pack_lhsT(U1)
    w4 = _pack_lhsT(U4)

    def blocks(U):  # (2, 2, 3, 128, 128)
        return np.stack([
            np.stack([_pack_lhsT(U[j * 128:(j + 1) * 128, k * 128:(k + 1) * 128])
                      for k in (0, 1)])
            for j in (0, 1)])

    w2 = blocks(U2)
    w3 = blocks(U3)
    ident = np.eye(128, dtype=np.float32)
    return {"w1": w1, "w2": w2, "w3": w3, "w4": w4, "ident": ident}


# ---------------------------------------------------------------------------
# Bass kernel builder
# ---------------------------------------------------------------------------

def build_nc(use_f32r=USE_F32R):
    nc = bacc.Bacc()
    SD_DRAM = F32R if use_f32r else F32

    st = nc.declare_dram_parameter("state", [2, 1 << 21], F32, isOutput=False)
    w1 = nc.declare_dram_parameter("w1", [3, 128, 128], F32, isOutput=False)
    w2 = nc.declare_dram_parameter("w2", [2, 2, 3, 128, 128], F32, isOutput=False)
    w3 = nc.declare_dram_parameter("w3", [2, 2, 3, 128, 128], F32, isOutput=False)
    w4 = nc.declare_dram_parameter("w4", [3, 128, 128], F32, isOutput=False)
    idn = nc.declare_dram_parameter("ident", [128, 128], F32, isOutput=False)
    out = nc.declare_dram_parameter("out", [2, 1 << 21], F32, isOutput=True)

    # AllToAll bounce buffers, split in four quarters by (q3,q4) so the
    # collective overlaps P3/P4 compute. bf16 wire format halves the bytes
    # (precision budget: f32r matmuls already give ~3e-4; bf16 transport of
    # the state adds ~4e-3, well under the 2e-2 gate).
    a2a_in = [nc.dram_tensor(f"a2a_in{h}", [8, 2, 128, 512], BF16)
              for h in range(4)]
    a2a_out = [nc.dram_tensor(f"a2a_out{h}", [8, 2, 128, 512], BF16)
               for h in range(4)]

    SD = F32R if use_f32r else F32  # SBUF/bounce storage dtype

    def ldma(out_ap, in_ap):
        # DRAM f32 -> SBUF f32r needs the SWDGE cast path
        if use_f32r:
            nc.gpsimd.dma_start(out=out_ap, in_=in_ap)
        else:
            nc.sync.dma_start(out=out_ap, in_=in_ap)

    with tile.TileContext(nc, num_cores=N_CORES) as tc:
        with tc.tile_pool(name="state", bufs=1) as sp, \
             tc.tile_pool(name="wpool", bufs=1) as wp, \
             tc.tile_pool(name="mm", bufs=6, space="PSUM") as mmp, \
             tc.tile_pool(name="tr", bufs=2, space="PSUM") as trp:

            sre = sp.tile([128, 16384], SD, tag="sre")
            sim = sp.tile([128, 16384], SD, tag="sim")
            # bf16 staging for the A2A wire (2 rotating quarter-slots)
            stg_re = sp.tile([128, 8192], BF16, tag="stg_re")
            stg_im = sp.tile([128, 8192], BF16, tag="stg_im")

            # ---- load weights ----
            def load_w3(dram_ap3, name):  # (3,128,128) -> 3 sbuf tiles
                ts = []
                for i in range(3):
                    t = wp.tile([128, 128], SD, tag=f"{name}_{i}")
                    ldma(t[:], dram_ap3[i])
                    ts.append(t)
                return ts

            w1t = load_w3(w1, "w1")
            w4t = load_w3(w4, "w4")
            w2t = [[load_w3(w2[j, k], f"w2_{j}{k}") for k in (0, 1)] for j in (0, 1)]
            w3t = [[load_w3(w3[j, k], f"w3_{j}{k}") for k in (0, 1)] for j in (0, 1)]
            idt = wp.tile([128, 128], SD, tag="ident")
            ldma(idt[:], idn[:])

            # ---- load state:  partitions (q0..q6), free (q7..q20) ----
            st_v = [st[pl].rearrange("(p f) -> p f", p=128) for pl in (0, 1)]
            for c in range(8):
                for pl, s in ((0, sre), (1, sim)):
                    ldma(s[:, c * 2048:(c + 1) * 2048],
                         st_v[pl][:, c * 2048:(c + 1) * 2048])

            planes = ((sre, sim))

            def cmul_into(pre, pim, W, xre, xim, start, stop=False):
                """pre += re(U)@xre - im(U)@xim ; pim += im(U)@xre + re(U)@xim
                W = [A, B, Bn] lhsT tiles."""
                A, B, Bn = W
                nc.tensor.matmul(pre[:], (A[:]), (xre), start=start, stop=False)
                nc.tensor.matmul(pim[:], (A[:]), (xim), start=start, stop=False)
                nc.tensor.matmul(pim[:], (B[:]), (xre), start=False, stop=stop)
                nc.tensor.matmul(pre[:], (Bn[:]), (xim), start=False, stop=stop)

            # ---- P1: chunk on partitions (q0..q6) ----
            # tiles processed in pairs, matmuls grouped by weight so the
            # stationary operand reloads 3x per 8 matmuls instead of 8x
            def simple_pass_pair(W, t, u):
                A, B, Bn = W
                ps = []
                xs = []
                for tt in (t, u):
                    c0 = tt * 512
                    pre = mmp.tile([128, 512], F32, tag="mm")
                    pim = mmp.tile([128, 512], F32, tag="mm")
                    ps.append((pre, pim))
                    xs.append((sre[:, c0:c0 + 512], sim[:, c0:c0 + 512]))
                for i in (0, 1):
                    nc.tensor.matmul(ps[i][0][:], (A[:]), xs[i][0], start=True, stop=False)
                    nc.tensor.matmul(ps[i][1][:], (A[:]), xs[i][1], start=True, stop=False)
                for i in (0, 1):
                    nc.tensor.matmul(ps[i][1][:], (B[:]), xs[i][0], start=False, stop=True)
                for i in (0, 1):
                    nc.tensor.matmul(ps[i][0][:], (Bn[:]), xs[i][1], start=False, stop=True)
                for i, tt in enumerate((t, u)):
                    c0 = tt * 512
                    nc.vector.tensor_copy(sre[:, c0:c0 + 512], ps[i][0][:])
                    nc.scalar.copy(out=sim[:, c0:c0 + 512], in_=ps[i][1][:])

            for t in range(0, 32, 2):
                simple_pass_pair(w1t, t, t + 1)

            # ---- T1: transpose partitions (q0..q6) <-> free (q7..q13) ----
            # L1 free = (q7..q13)*128 + (q14..q20); window w = (q14..q20):
            # read col-set {a*128+w}, transpose, write back to same col-set,
            # giving L2: partitions (q7..q13), free = (q0..q6)*128 + (q14..q20).
            for si, s in enumerate((sre, sim)):
                sv = s[:].rearrange("p (a w) -> p a w", w=128)
                for w in range(128):
                    pt = trp.tile([128, 128], SD, tag="tr")
                    nc.tensor.transpose((pt[:]), (sv[:, :, w]), (idt[:]))
                    if (w + si) % 3 == 0:
                        nc.scalar.copy(out=sv[:, :, w], in_=pt[:])
                    else:
                        nc.vector.tensor_copy(sv[:, :, w], pt[:])

            # ---- P2: chunk [6..13]; partitions (q7..q13), q6 = free bit ----
            # L2 free = (q0..q6)*128 + (q14..q20); q6 = bit0 of the outer
            # index => columns alternate 128-blocks by q6.
            sre_v = sre[:].rearrange("p (o q c) -> p o q c", q=2, c=128)
            sim_v = sim[:].rearrange("p (o q c) -> p o q c", q=2, c=128)
            for t in [x for x in range(0, 16, 2)] + [x for x in range(1, 16, 2)]:
                o0 = t * 4
                xr = [sre_v[:, o0:o0 + 4, k, :] for k in (0, 1)]
                xi = [sim_v[:, o0:o0 + 4, k, :] for k in (0, 1)]
                ps = []
                for j in (0, 1):
                    pre = mmp.tile([128, 512], F32, tag="mm")
                    pim = mmp.tile([128, 512], F32, tag="mm")
                    cmul_into(pre, pim, w2t[j][0], xr[0], xi[0], start=True)
                    cmul_into(pre, pim, w2t[j][1], xr[1], xi[1], start=False, stop=True)
                    ps.append((pre, pim))
                for j in (0, 1):
                    pre, pim = ps[j]
                    nc.vector.tensor_copy(sre_v[:, o0:o0 + 4, j, :], pre[:])
                    nc.scalar.copy(out=sim_v[:, o0:o0 + 4, j, :], in_=pim[:])

            # ---- T2: transpose partitions (q7..q13) <-> free (q14..q20) ----
            # window o = (q0..q6): read contiguous block [o*128, o*128+128),
            # write back contiguous, giving L3: partitions (q14..q20),
            # free = (q0..q6)*128 + (q7..q13) = (q0..q13) natural.
            # PSUM col j = (q7..q12)*2 + q13; write back with q13 at
            # stride 64 so P3's k13-halves are contiguous 64-elem runs.
            p3_order_t = [t for v in range(4) for t in range(32) if (t & 3) == v]
            t2_order = [o for t in p3_order_t for o in range(4 * t, 4 * t + 4)]
            for si, s in enumerate((sre, sim)):
                sv2 = s[:].rearrange("p (o k c) -> p o k c", k=2, c=64)
                for o in t2_order:
                    pt = trp.tile([128, 128], SD, tag="tr")
                    nc.tensor.transpose(
                        (pt[:]), (s[:, o * 128:o * 128 + 128]), (idt[:]))
                    ptv = pt[:].rearrange("p (c k) -> p k c", k=2)
                    if (o + si) % 3 == 0:
                        nc.scalar.copy(out=sv2[:, o], in_=ptv)
                    else:
                        nc.vector.tensor_copy(sv2[:, o], ptv)

            # ---- P3: chunk [13..20]; partitions (q14..q20), q13 = free bit0 ----
            sre_w = sre[:].rearrange("p (o k b) -> p o k b", k=2, b=64)
            sim_w = sim[:].rearrange("p (o k b) -> p o k b", k=2, b=64)

            # staging views: slot = quarter v % 2, each slot 4096 cols
            stg_re_w = stg_re[:].rearrange("p (s o k b) -> p s o k b",
                                           s=2, k=2, b=64)
            stg_im_w = stg_im[:].rearrange("p (s o k b) -> p s o k b",
                                           s=2, k=2, b=64)

            def p3_tile(t):
                o0 = t * 4
                slot = (t & 3) % 2
                so = (t >> 2) * 4  # position of this tile inside its slot
                xr = [sre_w[:, o0:o0 + 4, k, :] for k in (0, 1)]
                xi = [sim_w[:, o0:o0 + 4, k, :] for k in (0, 1)]
                ps = []
                for j in (0, 1):
                    pre = mmp.tile([128, 256], F32, tag="mm")
                    pim = mmp.tile([128, 256], F32, tag="mm")
                    cmul_into(pre, pim, w3t[j][0], xr[0], xi[0], start=True)
                    cmul_into(pre, pim, w3t[j][1], xr[1], xi[1], start=False, stop=True)
                    ps.append((pre, pim))
                for j in (0, 1):
                    pre, pim = ps[j]
                    nc.vector.tensor_copy(stg_re_w[:, slot, so:so + 4, j, :], pre[:])
                    nc.scalar.copy(out=stg_im_w[:, slot, so:so + 4, j, :], in_=pim[:])

            def stage_quarter(v):
                # staging slot (part q14..q20, [b=(q0q1q2), f]) -> a2a_in[v]
                slot = v % 2
                for pl, s in ((0, stg_re), (1, stg_im)):
                    sv = s[:].rearrange("p (t b f) -> p t b f", t=2, b=8)
                    nc.sync.dma_start(
                        out=a2a_in[v][:, pl].rearrange("b p f -> p b f"),
                        in_=sv[:, slot, :, :])
                nc.gpsimd.collective_compute(
                    "AllToAll",
                    mybir.AluOpType.bypass,
                    replica_groups=[list(range(N_CORES))],
                    ins=[a2a_in[v].ap().opt()],
                    outs=[a2a_out[v].ap().opt()],
                )

            def readback_quarter(v):
                # a2a_out[v][s3, pl, (h3,m), f] -> partitions s3*16+m,
                # free = h3*2048 + v*512 + f   (bf16 -> f32r cast via SWDGE)
                for s3 in range(8):
                    for pl, s in ((0, sre), (1, sim)):
                        sv = (s[s3 * 16:(s3 + 1) * 16, :]
                              .rearrange("m (h3 g f) -> m h3 g f", h3=8, g=4))
                        nc.gpsimd.dma_start(
                            out=sv[:, :, v, :],
                            in_=a2a_out[v][s3, pl]
                                .rearrange("(h3 m) f -> m h3 f", m=16))

            # P3 tile t = (q0..q4); quarter v = (q3,q4) = t & 3
            for v in range(4):
                for t in range(32):
                    if t & 3 == v:
                        p3_tile(t)
                stage_quarter(v)


            # ---- P4: chunk [20..23] on partitions (q21,q22,q23,q17..q20) ----
            # P4 tile t covers free [512t, 512t+512) = fixes
            # (q14q15q16, q3, q4); quarter v = t & 3
            for v in range(4):
                readback_quarter(v)
                tiles = [t for t in range(32) if (t & 3) == v]
                for i in range(0, 8, 2):
                    t, u = tiles[i], tiles[i + 1]
                    simple_pass_pair(w4t, t, u)
                    for tt in (t, u):
                        c0 = tt * 512
                        for pl, s in ((0, sre), (1, sim)):
                            ov = out[pl].rearrange("(p f) -> p f", p=128)
                            nc.sync.dma_start(
                                out=ov[:, c0:c0 + 512],
                                in_=(s[:, c0:c0 + 512].bitcast(F32)
                                     if use_f32r else s[:, c0:c0 + 512]))

    return nc


# ---------------------------------------------------------------------------
# Host wrapper
# ---------------------------------------------------------------------------

TRACE = False          # set by test harnesses to capture a profile
LAST_EXEC_NS = None
LAST_RESULTS = None


def kernel(state, gates1, gates2):
    global LAST_EXEC_NS, LAST_RESULTS
    state = np.ascontiguousarray(np.asarray(state, dtype=np.float32))
    weights = build_weights(np.asarray(gates1, dtype=np.float32),
                            np.asarray(gates2, dtype=np.float32))

    # shard over (q21,q22,q23) = index mod 8
    shards = np.ascontiguousarray(
        state.reshape(2, 1 << 21, 8).transpose(2, 0, 1))

    nc = build_nc()
    if not nc.is_finalized():
        nc.finalize()
    in_maps = [dict(weights, state=shards[d]) for d in range(N_CORES)]
    res = run_bass_kernel_spmd(nc, in_maps, core_ids=list(range(N_CORES)),
                               trace=TRACE)
    LAST_EXEC_NS = res.exec_time_ns
    LAST_RESULTS = res

    return unshard([res.results[d]["out"] for d in range(N_CORES)])


def unshard(outs):
    # core d holds (q0,q1,q2) = d;
    # out layout = [plane][s=(q21..q23), m=(q17..q20)][h3=(q14..q16)]
    #              [a=(q3..q6)][k=(q13)][c=(q7..q12)]
    full = np.empty((2, 8, 16, 64, 2, 8, 16, 8), dtype=np.float32)
    for d in range(N_CORES):
        od = np.asarray(outs[d]).reshape(2, 8, 16, 8, 16, 2, 64)
        full[:, d] = od.transpose(0, 4, 6, 5, 3, 2, 1)
    return full.reshape(2, 1 << 24)


if __name__ == "__main__":
    rng = np.random.default_rng(0)
    state = rng.standard_normal((2, 1 << 24)).astype(np.float32)
    g1 = rng.standard_normal((24, 2, 2, 2)).astype(np.float32)
    g2 = rng.standard_normal((23, 2, 4, 4)).astype(np.float32)
    out = kernel(state, g1, g2)
    print(out.shape, out.dtype)


# revision 21
# speedup vs baseline: 1.4176x; 1.0348x over previous
"""Trainium2 Bass kernel for nn_Circuit_19275813225041.

24-qubit state-vector simulation: one layer of single-qubit gates on every
qubit, then a ladder of two-qubit gates on neighboring pairs (q, q+1),
q = 0..22, on a 2^24 complex state stored as (2, 2^24) float32 (re, im).

Strategy (8 NeuronCores):
  - Qubit q <-> bit q of the state index, bit 0 = MSB.
  - Shard the state over the 3 LSB qubits (q21,q22,q23): core d holds
    amplitudes with index % 8 == d (state-vector slicing).
  - Gates are fused on the host into 4 big chunk matrices:
      U1: 128x128 on qubits [0..6]    (singles 0..6, ladder (0,1)..(5,6))
      U2: 256x256 on qubits [6..13]   (singles 7..13, ladder (6,7)..(12,13))
      U3: 256x256 on qubits [13..20]  (singles 14..20, ladder (13,14)..(19,20))
      U4: 128x128 on qubits [21,22,23,17..20] (singles 21..23, ladder
          (20,21),(21,22),(22,23), identity on q17..q19)
    (The reference's _apply_gate has a permutation quirk for the 2-qubit
    gate at q=1 -- its "inverse" transpose applies perm again, which for
    q=1 is a 3-cycle.  This adds a relabeling permutation on qubits
    (0,1,2) right after that gate; it is folded into U1.)
  - Each core applies U1..U3 to its local 2^21 state via TensorE matmuls,
    with PE transposes rotating 7-bit groups through the partition axis and
    2-term PSUM accumulation handling the chunk boundary bit (q6, q13).
  - One AllToAll swaps qubits (q0,q1,q2) <-> (q21,q22,q23) across cores so
    the final chunk U4 applies locally; output is returned sharded over
    (q0,q1,q2) and reassembled on the host.
"""

import numpy as np

import concourse.bass as bass
import concourse.bacc as bacc
import concourse.mybir as mybir
import concourse.tile as tile
from concourse.bass_utils import run_bass_kernel_spmd

F32 = mybir.dt.float32
F32R = mybir.dt.float32r
BF16 = mybir.dt.bfloat16

USE_F32R = True   # fast fp32 matmul mode (reduced precision)
N_CORES = 8


# ---------------------------------------------------------------------------
# Host-side gate fusion
# ---------------------------------------------------------------------------

def _embed_gate(mat, qubits, group):
    """Embed `mat` acting on `qubits` (MSB-first) into the space indexed by
    `group` (list of qubits, group[0] = MSB of the index)."""
    g = len(group)
    k = len(qubits)
    pos = [group.index(q) for q in qubits]
    rest = [i for i in range(g) if i not in pos]
    U = np.zeros((1 << g, 1 << g), dtype=np.complex128)
    for r in range(1 << len(rest)):
        base = 0
        for bi, p in enumerate(rest):
            if (r >> (len(rest) - 1 - bi)) & 1:
                base |= 1 << (g - 1 - p)
        for a in range(1 << k):
            ia = base
            for bi, p in enumerate(pos):
                if (a >> (k - 1 - bi)) & 1:
                    ia |= 1 << (g - 1 - p)
            for b in range(1 << k):
                ib = base
                for bi, p in enumerate(pos):
                    if (b >> (k - 1 - bi)) & 1:
                        ib |= 1 << (g - 1 - p)
                U[ia, ib] = mat[a, b]
    return U


def _quirk_P():
    # reference._apply_gate on [1,2]: the un-permute uses perm (a 3-cycle)
    # instead of its inverse => extra relabeling on qubits (0,1,2):
    # new (b0,b1,b2) = (old b2, old b0, old b1).
    P = np.zeros((8, 8), dtype=np.complex128)
    for b0 in range(2):
        for b1 in range(2):
            for b2 in range(2):
                P[(b2 << 2) | (b0 << 1) | b1, (b0 << 2) | (b1 << 1) | b2] = 1
    return P


def _fuse(ops, group):
    U = np.eye(1 << len(group), dtype=np.complex128)
    for mat, qb in ops:
        U = _embed_gate(mat, qb, group) @ U
    return U


def build_chunk_matrices(gates1, gates2):
    g1 = gates1[:, 0].astype(np.float64) + 1j * gates1[:, 1].astype(np.float64)
    g2 = gates2[:, 0].astype(np.float64) + 1j * gates2[:, 1].astype(np.float64)

    ops1 = [(g1[q], [q]) for q in range(0, 7)]
    ops1 += [(g2[0], [0, 1]), (g2[1], [1, 2]), (_quirk_P(), [0, 1, 2])]
    ops1 += [(g2[q], [q, q + 1]) for q in range(2, 6)]
    U1 = _fuse(ops1, list(range(0, 7)))

    ops2 = [(g1[q], [q]) for q in range(7, 14)]
    ops2 += [(g2[q], [q, q + 1]) for q in range(6, 13)]
    U2 = _fuse(ops2, list(range(6, 14)))  # q6 = MSB of the 256 index

    ops3 = [(g1[q], [q]) for q in range(14, 21)]
    ops3 += [(g2[q], [q, q + 1]) for q in range(13, 20)]
    U3 = _fuse(ops3, list(range(13, 21)))  # q13 = MSB

    ops4 = [(g1[q], [q]) for q in range(21, 24)]
    ops4 += [(g2[q], [q, q + 1]) for q in range(20, 23)]
    # partition index on the device = s*16 + m, s = (q21,q22,q23), m = (q17..q20)
    U4 = _fuse(ops4, [21, 22, 23, 17, 18, 19, 20])

    return U1, U2, U3, U4


def _pack_lhsT(U):
    """lhsT components for out = U @ x (complex):  A = re(U)^T, B = im(U)^T,
    Bn = -im(U)^T, stacked (3, n, n) float32."""
    return np.stack([U.real.T, U.imag.T, -U.imag.T]).astype(np.float32)


def build_weights(gates1, gates2):
    U1, U2, U3, U4 = build_chunk_matrices(gates1, gates2)
    w1 = _pack_lhsT(U1)
    w4 = _pack_lhsT(U4)

    def blocks(U):  # (2, 2, 3, 128, 128)
        return np.stack([
            np.stack([_pack_lhsT(U[j * 128:(j + 1) * 128, k * 128:(k + 1) * 128])
                      for k in (0, 1)])
            for j in (0, 1)])

    w2 = blocks(U2)
    w3 = blocks(U3)
    ident = np.eye(128, dtype=np.float32)
    return {"w1": w1, "w2": w2, "w3": w3, "w4": w4, "ident": ident}


# ---------------------------------------------------------------------------
# Bass kernel builder
# ---------------------------------------------------------------------------

def build_nc(use_f32r=USE_F32R):
    nc = bacc.Bacc()
    SD_DRAM = F32R if use_f32r else F32

    st = nc.declare_dram_parameter("state", [2, 1 << 21], F32, isOutput=False)
    w1 = nc.declare_dram_parameter("w1", [3, 128, 128], F32, isOutput=False)
    w2 = nc.declare_dram_parameter("w2", [2, 2, 3, 128, 128], F32, isOutput=False)
    w3 = nc.declare_dram_parameter("w3", [2, 2, 3, 128, 128], F32, isOutput=False)
    w4 = nc.declare_dram_parameter("w4", [3, 128, 128], F32, isOutput=False)
    idn = nc.declare_dram_parameter("ident", [128, 128], F32, isOutput=False)
    out = nc.declare_dram_parameter("out", [2, 1 << 21], F32, isOutput=True)

    # AllToAll bounce buffers, split in four quarters by (q3,q4) so the
    # collective overlaps P3/P4 compute. bf16 wire format halves the bytes
    # (precision budget: f32r matmuls already give ~3e-4; bf16 transport of
    # the state adds ~4e-3, well under the 2e-2 gate).
    a2a_in = [nc.dram_tensor(f"a2a_in{h}", [8, 2, 128, 512], BF16)
              for h in range(4)]
    a2a_out = [nc.dram_tensor(f"a2a_out{h}", [8, 2, 128, 512], BF16)
               for h in range(4)]

    SD = F32R if use_f32r else F32  # SBUF/bounce storage dtype

    def ldma(out_ap, in_ap):
        # DRAM f32 -> SBUF f32r needs the SWDGE cast path
        if use_f32r:
            nc.gpsimd.dma_start(out=out_ap, in_=in_ap)
        else:
            nc.sync.dma_start(out=out_ap, in_=in_ap)

    with tile.TileContext(nc, num_cores=N_CORES) as tc:
        with tc.tile_pool(name="state", bufs=1) as sp, \
             tc.tile_pool(name="wpool", bufs=1) as wp, \
             tc.tile_pool(name="mm", bufs=6, space="PSUM") as mmp, \
             tc.tile_pool(name="tr", bufs=2, space="PSUM") as trp:

            sre = sp.tile([128, 16384], SD, tag="sre")
            sim = sp.tile([128, 16384], SD, tag="sim")
            # bf16 staging for the A2A wire (2 rotating quarter-slots)
            stg_re = sp.tile([128, 8192], BF16, tag="stg_re")
            stg_im = sp.tile([128, 8192], BF16, tag="stg_im")

            # ---- load weights ----
            def load_w3(dram_ap3, name):  # (3,128,128) -> 3 sbuf tiles
                ts = []
                for i in range(3):
                    t = wp.tile([128, 128], SD, tag=f"{name}_{i}")
                    ldma(t[:], dram_ap3[i])
                    ts.append(t)
                return ts

            w1t = load_w3(w1, "w1")
            w4t = load_w3(w4, "w4")
            w2t = [[load_w3(w2[j, k], f"w2_{j}{k}") for k in (0, 1)] for j in (0, 1)]
            w3t = [[load_w3(w3[j, k], f"w3_{j}{k}") for k in (0, 1)] for j in (0, 1)]
            idt = wp.tile([128, 128], SD, tag="ident")
            ldma(idt[:], idn[:])

            # ---- load state:  partitions (q0..q6), free (q7..q20) ----
            st_v = [st[pl].rearrange("(p f) -> p f", p=128) for pl in (0, 1)]
            for c in range(8):
                for pl, s in ((0, sre), (1, sim)):
                    ldma(s[:, c * 2048:(c + 1) * 2048],
                         st_v[pl][:, c * 2048:(c + 1) * 2048])

            planes = ((sre, sim))

            def cmul_into(pre, pim, W, xre, xim, start, stop=False):
                """pre += re(U)@xre - im(U)@xim ; pim += im(U)@xre + re(U)@xim
                W = [A, B, Bn] lhsT tiles."""
                A, B, Bn = W
                nc.tensor.matmul(pre[:], (A[:]), (xre), start=start, stop=False)
                nc.tensor.matmul(pim[:], (A[:]), (xim), start=start, stop=False)
                nc.tensor.matmul(pim[:], (B[:]), (xre), start=False, stop=stop)
                nc.tensor.matmul(pre[:], (Bn[:]), (xim), start=False, stop=stop)

            # ---- P1: chunk on partitions (q0..q6) ----
            # tiles processed in pairs, matmuls grouped by weight so the
            # stationary operand reloads 3x per 8 matmuls instead of 8x
            def simple_pass_pair(W, t, u):
                A, B, Bn = W
                ps = []
                xs = []
                for tt in (t, u):
                    c0 = tt * 512
                    pre = mmp.tile([128, 512], F32, tag="mm")
                    pim = mmp.tile([128, 512], F32, tag="mm")
                    ps.append((pre, pim))
                    xs.append((sre[:, c0:c0 + 512], sim[:, c0:c0 + 512]))
                for i in (0, 1):
                    nc.tensor.matmul(ps[i][0][:], (A[:]), xs[i][0], start=True, stop=False)
                    nc.tensor.matmul(ps[i][1][:], (A[:]), xs[i][1], start=True, stop=False)
                for i in (0, 1):
                    nc.tensor.matmul(ps[i][1][:], (B[:]), xs[i][0], start=False, stop=True)
                for i in (0, 1):
                    nc.tensor.matmul(ps[i][0][:], (Bn[:]), xs[i][1], start=False, stop=True)
                for i, tt in enumerate((t, u)):
                    c0 = tt * 512
                    nc.vector.tensor_copy(sre[:, c0:c0 + 512], ps[i][0][:])
                    nc.scalar.copy(out=sim[:, c0:c0 + 512], in_=ps[i][1][:])

            for t in range(0, 32, 2):
                simple_pass_pair(w1t, t, t + 1)

            # ---- T1: transpose partitions (q0..q6) <-> free (q7..q13) ----
            # L1 free = (q7..q13)*128 + (q14..q20); window w = (q14..q20):
            # read col-set {a*128+w}, transpose, write back to same col-set,
            # giving L2: partitions (q7..q13), free = (q0..q6)*128 + (q14..q20).
            for si, s in enumerate((sre, sim)):
                sv = s[:].rearrange("p (a w) -> p a w", w=128)
                for w in range(128):
                    pt = trp.tile([128, 128], SD, tag="tr")
                    nc.tensor.transpose((pt[:]), (sv[:, :, w]), (idt[:]))
                    if (w + si) % 3 == 0:
                        nc.scalar.copy(out=sv[:, :, w], in_=pt[:])
                    else:
                        nc.vector.tensor_copy(sv[:, :, w], pt[:])

            # ---- P2: chunk [6..13]; partitions (q7..q13), q6 = free bit ----
            # L2 free = (q0..q6)*128 + (q14..q20); q6 = bit0 of the outer
            # index => columns alternate 128-blocks by q6.
            sre_v = sre[:].rearrange("p (o q c) -> p o q c", q=2, c=128)
            sim_v = sim[:].rearrange("p (o q c) -> p o q c", q=2, c=128)

            def p2_tile(t):
                o0 = t * 4
                xr = [sre_v[:, o0:o0 + 4, k, :] for k in (0, 1)]
                xi = [sim_v[:, o0:o0 + 4, k, :] for k in (0, 1)]
                ps = []
                for j in (0, 1):
                    pre = mmp.tile([128, 512], F32, tag="mm")
                    pim = mmp.tile([128, 512], F32, tag="mm")
                    cmul_into(pre, pim, w2t[j][0], xr[0], xi[0], start=True)
                    cmul_into(pre, pim, w2t[j][1], xr[1], xi[1], start=False, stop=True)
                    ps.append((pre, pim))
                for j in (0, 1):
                    pre, pim = ps[j]
                    nc.vector.tensor_copy(sre_v[:, o0:o0 + 4, j, :], pre[:])
                    nc.scalar.copy(out=sim_v[:, o0:o0 + 4, j, :], in_=pim[:])

            # ---- T2: transpose partitions (q7..q13) <-> free (q14..q20) ----
            # window o = (q0..q6): read contiguous block [o*128, o*128+128),
            # write back with q13 at stride 64 (free = o*128 + q13*64 +
            # (q7..q12)) so P3's k13-halves are contiguous 64-elem runs.
            # L3: partitions (q14..q20), free as above.
            sv2s = [s[:].rearrange("p (o k c) -> p o k c", k=2, c=64)
                    for s in (sre, sim)]

            def t2_block(o):
                for si, s in enumerate((sre, sim)):
                    pt = trp.tile([128, 128], SD, tag="tr")
                    nc.tensor.transpose(
                        (pt[:]), (s[:, o * 128:o * 128 + 128]), (idt[:]))
                    ptv = pt[:].rearrange("p (c k) -> p k c", k=2)
                    if (o + si) % 3 == 0:
                        nc.scalar.copy(out=sv2s[si][:, o], in_=ptv)
                    else:
                        nc.vector.tensor_copy(sv2s[si][:, o], ptv)

            # ---- P3: chunk [13..20]; partitions (q14..q20), q13 on free ----
            sre_w = sre[:].rearrange("p (o k b) -> p o k b", k=2, b=64)
            sim_w = sim[:].rearrange("p (o k b) -> p o k b", k=2, b=64)

            # staging views: slot = quarter v % 2, each slot 4096 cols
            stg_re_w = stg_re[:].rearrange("p (s o k b) -> p s o k b",
                                           s=2, k=2, b=64)
            stg_im_w = stg_im[:].rearrange("p (s o k b) -> p s o k b",
                                           s=2, k=2, b=64)

            def p3_tile(t):
                o0 = t * 4
                slot = (t & 3) % 2
                so = (t >> 2) * 4  # position of this tile inside its slot
                xr = [sre_w[:, o0:o0 + 4, k, :] for k in (0, 1)]
                xi = [sim_w[:, o0:o0 + 4, k, :] for k in (0, 1)]
                ps = []
                for j in (0, 1):
                    pre = mmp.tile([128, 256], F32, tag="mm")
                    pim = mmp.tile([128, 256], F32, tag="mm")
                    cmul_into(pre, pim, w3t[j][0], xr[0], xi[0], start=True)
                    cmul_into(pre, pim, w3t[j][1], xr[1], xi[1], start=False, stop=True)
                    ps.append((pre, pim))
                for j in (0, 1):
                    pre, pim = ps[j]
                    nc.vector.tensor_copy(stg_re_w[:, slot, so:so + 4, j, :], pre[:])
                    nc.scalar.copy(out=stg_im_w[:, slot, so:so + 4, j, :], in_=pim[:])

            def stage_quarter(v):
                # staging slot (part q14..q20, [b=(q0q1q2), f]) -> a2a_in[v]
                slot = v % 2
                for pl, s in ((0, stg_re), (1, stg_im)):
                    sv = s[:].rearrange("p (t b f) -> p t b f", t=2, b=8)
                    nc.sync.dma_start(
                        out=a2a_in[v][:, pl].rearrange("b p f -> p b f"),
                        in_=sv[:, slot, :, :])
                nc.gpsimd.collective_compute(
                    "AllToAll",
                    mybir.AluOpType.bypass,
                    replica_groups=[list(range(N_CORES))],
                    ins=[a2a_in[v].ap().opt()],
                    outs=[a2a_out[v].ap().opt()],
                )

            def readback_quarter(v):
                # a2a_out[v][s3, pl, (h3,m), f] -> partitions s3*16+m,
                # free = h3*2048 + v*512 + f   (bf16 -> f32r cast via SWDGE)
                for s3 in range(8):
                    for pl, s in ((0, sre), (1, sim)):
                        sv = (s[s3 * 16:(s3 + 1) * 16, :]
                              .rearrange("m (h3 g f) -> m h3 g f", h3=8, g=4))
                        nc.gpsimd.dma_start(
                            out=sv[:, :, v, :],
                            in_=a2a_out[v][s3, pl]
                                .rearrange("(h3 m) f -> m h3 f", m=16))

            # ---- emission order: pipeline the A2A train under compute ----
            # P2 even tiles cover the o-ranges P3 quarters 0/1 need;
            # odd tiles cover quarters 2/3.
            p3_q = [[t for t in range(32) if (t & 3) == v] for v in range(4)]
            t2_for = lambda ts: [o for t in ts for o in range(4 * t, 4 * t + 4)]

            for t in range(0, 16, 2):
                p2_tile(t)
            for o in t2_for(p3_q[0] + p3_q[1]):
                t2_block(o)
            for t in p3_q[0]:
                p3_tile(t)
            stage_quarter(0)
            for t in p3_q[1]:
                p3_tile(t)
            stage_quarter(1)
            for t in range(1, 16, 2):
                p2_tile(t)
            for o in t2_for(p3_q[2] + p3_q[3]):
                t2_block(o)
            for t in p3_q[2]:
                p3_tile(t)
            stage_quarter(2)
            for t in p3_q[3]:
                p3_tile(t)
            stage_quarter(3)

            # P4 tile t covers free [512t, 512t+512) = fixes
            # (q14q15q16, q3, q4); quarter v = t & 3
            for v in range(4):
                readback_quarter(v)
                tiles = [t for t in range(32) if (t & 3) == v]
                for i in range(0, 8, 2):
                    t, u = tiles[i], tiles[i + 1]
                    simple_pass_pair(w4t, t, u)
                    for tt in (t, u):
                        c0 = tt * 512
                        for pl, s in ((0, sre), (1, sim)):
                            ov = out[pl].rearrange("(p f) -> p f", p=128)
                            nc.sync.dma_start(
                                out=ov[:, c0:c0 + 512],
                                in_=(s[:, c0:c0 + 512].bitcast(F32)
                                     if use_f32r else s[:, c0:c0 + 512]))

    return nc


# ---------------------------------------------------------------------------
# Host wrapper
# ---------------------------------------------------------------------------

TRACE = False          # set by test harnesses to capture a profile
LAST_EXEC_NS = None
LAST_RESULTS = None


def kernel(state, gates1, gates2):
    global LAST_EXEC_NS, LAST_RESULTS
    state = np.ascontiguousarray(np.asarray(state, dtype=np.float32))
    weights = build_weights(np.asarray(gates1, dtype=np.float32),
                            np.asarray(gates2, dtype=np.float32))

    # shard over (q21,q22,q23) = index mod 8
    shards = np.ascontiguousarray(
        state.reshape(2, 1 << 21, 8).transpose(2, 0, 1))

    nc = build_nc()
    if not nc.is_finalized():
        nc.finalize()
    in_maps = [dict(weights, state=shards[d]) for d in range(N_CORES)]
    res = run_bass_kernel_spmd(nc, in_maps, core_ids=list(range(N_CORES)),
                               trace=TRACE)
    LAST_EXEC_NS = res.exec_time_ns
    LAST_RESULTS = res

    return unshard([res.results[d]["out"] for d in range(N_CORES)])


def unshard(outs):
    # core d holds (q0,q1,q2) = d;
    # out layout = [plane][s=(q21..q23), m=(q17..q20)][h3=(q14..q16)]
    #              [a=(q3..q6)][k=(q13)][c=(q7..q12)]
    full = np.empty((2, 8, 16, 64, 2, 8, 16, 8), dtype=np.float32)
    for d in range(N_CORES):
        od = np.asarray(outs[d]).reshape(2, 8, 16, 8, 16, 2, 64)
        full[:, d] = od.transpose(0, 4, 6, 5, 3, 2, 1)
    return full.reshape(2, 1 << 24)


if __name__ == "__main__":
    rng = np.random.default_rng(0)
    state = rng.standard_normal((2, 1 << 24)).astype(np.float32)
    g1 = rng.standard_normal((24, 2, 2, 2)).astype(np.float32)
    g2 = rng.standard_normal((23, 2, 4, 4)).astype(np.float32)
    out = kernel(state, g1, g2)
    print(out.shape, out.dtype)


# revision 24
# speedup vs baseline: 1.4905x; 1.0515x over previous
"""Trainium2 Bass kernel for nn_Circuit_19275813225041.

24-qubit state-vector simulation: one layer of single-qubit gates on every
qubit, then a ladder of two-qubit gates on neighboring pairs (q, q+1),
q = 0..22, on a 2^24 complex state stored as (2, 2^24) float32 (re, im).

Strategy (8 NeuronCores):
  - Qubit q <-> bit q of the state index, bit 0 = MSB.
  - Shard the state over the 3 LSB qubits (q21,q22,q23): core d holds
    amplitudes with index % 8 == d (state-vector slicing).
  - Gates are fused on the host into 4 big chunk matrices:
      U1: 128x128 on qubits [0..6]    (singles 0..6, ladder (0,1)..(5,6))
      U2: 256x256 on qubits [6..13]   (singles 7..13, ladder (6,7)..(12,13))
      U3: 256x256 on qubits [13..20]  (singles 14..20, ladder (13,14)..(19,20))
      U4: 128x128 on qubits [21,22,23,17..20] (singles 21..23, ladder
          (20,21),(21,22),(22,23), identity on q17..q19)
    (The reference's _apply_gate has a permutation quirk for the 2-qubit
    gate at q=1 -- its "inverse" transpose applies perm again, which for
    q=1 is a 3-cycle.  This adds a relabeling permutation on qubits
    (0,1,2) right after that gate; it is folded into U1.)
  - Each core applies U1..U3 to its local 2^21 state via TensorE matmuls,
    with PE transposes rotating 7-bit groups through the partition axis and
    2-term PSUM accumulation handling the chunk boundary bit (q6, q13).
  - One AllToAll swaps qubits (q0,q1,q2) <-> (q21,q22,q23) across cores so
    the final chunk U4 applies locally; output is returned sharded over
    (q0,q1,q2) and reassembled on the host.
"""

import numpy as np

import concourse.bass as bass
import concourse.bacc as bacc
import concourse.mybir as mybir
import concourse.tile as tile
from concourse.bass_utils import run_bass_kernel_spmd

F32 = mybir.dt.float32
F32R = mybir.dt.float32r
BF16 = mybir.dt.bfloat16

USE_F32R = True   # fast fp32 matmul mode (reduced precision)
N_CORES = 8


# ---------------------------------------------------------------------------
# Host-side gate fusion
# ---------------------------------------------------------------------------

def _embed_gate(mat, qubits, group):
    """Embed `mat` acting on `qubits` (MSB-first) into the space indexed by
    `group` (list of qubits, group[0] = MSB of the index)."""
    g = len(group)
    k = len(qubits)
    pos = [group.index(q) for q in qubits]
    rest = [i for i in range(g) if i not in pos]
    U = np.zeros((1 << g, 1 << g), dtype=np.complex128)
    for r in range(1 << len(rest)):
        base = 0
        for bi, p in enumerate(rest):
            if (r >> (len(rest) - 1 - bi)) & 1:
                base |= 1 << (g - 1 - p)
        for a in range(1 << k):
            ia = base
            for bi, p in enumerate(pos):
                if (a >> (k - 1 - bi)) & 1:
                    ia |= 1 << (g - 1 - p)
            for b in range(1 << k):
                ib = base
                for bi, p in enumerate(pos):
                    if (b >> (k - 1 - bi)) & 1:
                        ib |= 1 << (g - 1 - p)
                U[ia, ib] = mat[a, b]
    return U


def _quirk_P():
    # reference._apply_gate on [1,2]: the un-permute uses perm (a 3-cycle)
    # instead of its inverse => extra relabeling on qubits (0,1,2):
    # new (b0,b1,b2) = (old b2, old b0, old b1).
    P = np.zeros((8, 8), dtype=np.complex128)
    for b0 in range(2):
        for b1 in range(2):
            for b2 in range(2):
                P[(b2 << 2) | (b0 << 1) | b1, (b0 << 2) | (b1 << 1) | b2] = 1
    return P


def _fuse(ops, group):
    U = np.eye(1 << len(group), dtype=np.complex128)
    for mat, qb in ops:
        U = _embed_gate(mat, qb, group) @ U
    return U


def build_chunk_matrices(gates1, gates2):
    g1 = gates1[:, 0].astype(np.float64) + 1j * gates1[:, 1].astype(np.float64)
    g2 = gates2[:, 0].astype(np.float64) + 1j * gates2[:, 1].astype(np.float64)

    ops1 = [(g1[q], [q]) for q in range(0, 7)]
    ops1 += [(g2[0], [0, 1]), (g2[1], [1, 2]), (_quirk_P(), [0, 1, 2])]
    ops1 += [(g2[q], [q, q + 1]) for q in range(2, 6)]
    U1 = _fuse(ops1, list(range(0, 7)))

    ops2 = [(g1[q], [q]) for q in range(7, 14)]
    ops2 += [(g2[q], [q, q + 1]) for q in range(6, 13)]
    U2 = _fuse(ops2, list(range(6, 14)))  # q6 = MSB of the 256 index

    ops3 = [(g1[q], [q]) for q in range(14, 21)]
    ops3 += [(g2[q], [q, q + 1]) for q in range(13, 20)]
    U3 = _fuse(ops3, list(range(13, 21)))  # q13 = MSB

    ops4 = [(g1[q], [q]) for q in range(21, 24)]
    ops4 += [(g2[q], [q, q + 1]) for q in range(20, 23)]
    # partition index on the device = s*16 + m, s = (q21,q22,q23), m = (q17..q20)
    U4 = _fuse(ops4, [21, 22, 23, 17, 18, 19, 20])

    return U1, U2, U3, U4


def _pack_lhsT(U):
    """lhsT components for out = U @ x (complex):  A = re(U)^T, B = im(U)^T,
    Bn = -im(U)^T, stacked (3, n, n) float32."""
    return np.stack([U.real.T, U.imag.T, -U.imag.T]).astype(np.float32)


def build_weights(gates1, gates2):
    U1, U2, U3, U4 = build_chunk_matrices(gates1, gates2)
    w1 = _pack_lhsT(U1)
    w4 = _pack_lhsT(U4)

    def blocks(U):  # (2, 2, 3, 128, 128)
        return np.stack([
            np.stack([_pack_lhsT(U[j * 128:(j + 1) * 128, k * 128:(k + 1) * 128])
                      for k in (0, 1)])
            for j in (0, 1)])

    w2 = blocks(U2)
    w3 = blocks(U3)
    ident = np.eye(128, dtype=np.float32)
    return {"w1": w1, "w2": w2, "w3": w3, "w4": w4, "ident": ident}


# ---------------------------------------------------------------------------
# Bass kernel builder
# ---------------------------------------------------------------------------

def build_nc(use_f32r=USE_F32R):
    nc = bacc.Bacc()
    SD_DRAM = F32R if use_f32r else F32

    st = nc.declare_dram_parameter("state", [2, 1 << 21], F32, isOutput=False)
    w1 = nc.declare_dram_parameter("w1", [3, 128, 128], F32, isOutput=False)
    w2 = nc.declare_dram_parameter("w2", [2, 2, 3, 128, 128], F32, isOutput=False)
    w3 = nc.declare_dram_parameter("w3", [2, 2, 3, 128, 128], F32, isOutput=False)
    w4 = nc.declare_dram_parameter("w4", [3, 128, 128], F32, isOutput=False)
    idn = nc.declare_dram_parameter("ident", [128, 128], F32, isOutput=False)
    out = nc.declare_dram_parameter("out", [2, 1 << 21], F32, isOutput=True)

    # AllToAll bounce buffers, split in four quarters by (q3,q4) so the
    # collective overlaps P3/P4 compute. bf16 wire format halves the bytes
    # (precision budget: f32r matmuls already give ~3e-4; bf16 transport of
    # the state adds ~4e-3, well under the 2e-2 gate).
    a2a_in = [nc.dram_tensor(f"a2a_in{h}", [8, 2, 128, 512], BF16)
              for h in range(4)]
    a2a_out = [nc.dram_tensor(f"a2a_out{h}", [8, 2, 128, 512], BF16)
               for h in range(4)]

    SD = F32R if use_f32r else F32  # SBUF/bounce storage dtype

    def ldma(out_ap, in_ap):
        # DRAM f32 -> SBUF f32r needs the SWDGE cast path
        if use_f32r:
            nc.gpsimd.dma_start(out=out_ap, in_=in_ap)
        else:
            nc.sync.dma_start(out=out_ap, in_=in_ap)

    with tile.TileContext(nc, num_cores=N_CORES) as tc:
        with tc.tile_pool(name="state", bufs=1) as sp, \
             tc.tile_pool(name="wpool", bufs=1) as wp, \
             tc.tile_pool(name="mm", bufs=6, space="PSUM") as mmp, \
             tc.tile_pool(name="tr", bufs=2, space="PSUM") as trp, \
             tc.tile_pool(name="outp", bufs=8) as op:

            sre = sp.tile([128, 16384], SD, tag="sre")
            sim = sp.tile([128, 16384], SD, tag="sim")
            # bf16 staging for the A2A wire (2 rotating quarter-slots)
            stg_re = sp.tile([128, 8192], BF16, tag="stg_re")
            stg_im = sp.tile([128, 8192], BF16, tag="stg_im")

            # ---- load weights ----
            def load_w3(dram_ap3, name):  # (3,128,128) -> 3 sbuf tiles
                ts = []
                for i in range(3):
                    t = wp.tile([128, 128], SD, tag=f"{name}_{i}")
                    ldma(t[:], dram_ap3[i])
                    ts.append(t)
                return ts

            w1t = load_w3(w1, "w1")
            w4t = []
            for i in range(3):
                w4b = wp.tile([128, 128], BF16, tag=f"w4b_{i}")
                nc.gpsimd.dma_start(out=w4b[:], in_=w4[i])
                w4t.append(w4b)
            w2t = [[load_w3(w2[j, k], f"w2_{j}{k}") for k in (0, 1)] for j in (0, 1)]
            w3t = [[load_w3(w3[j, k], f"w3_{j}{k}") for k in (0, 1)] for j in (0, 1)]
            idt = wp.tile([128, 128], SD, tag="ident")
            ldma(idt[:], idn[:])

            # ---- load state:  partitions (q0..q6), free (q7..q20) ----
            st_v = [st[pl].rearrange("(p f) -> p f", p=128) for pl in (0, 1)]
            for c in range(8):
                for pl, s in ((0, sre), (1, sim)):
                    ldma(s[:, c * 2048:(c + 1) * 2048],
                         st_v[pl][:, c * 2048:(c + 1) * 2048])

            planes = ((sre, sim))

            def cmul_into(pre, pim, W, xre, xim, start, stop=False):
                """pre += re(U)@xre - im(U)@xim ; pim += im(U)@xre + re(U)@xim
                W = [A, B, Bn] lhsT tiles."""
                A, B, Bn = W
                nc.tensor.matmul(pre[:], (A[:]), (xre), start=start, stop=False)
                nc.tensor.matmul(pim[:], (A[:]), (xim), start=start, stop=False)
                nc.tensor.matmul(pim[:], (B[:]), (xre), start=False, stop=stop)
                nc.tensor.matmul(pre[:], (Bn[:]), (xim), start=False, stop=stop)

            # ---- P1: chunk on partitions (q0..q6) ----
            # tiles processed in pairs, matmuls grouped by weight so the
            # stationary operand reloads 3x per 8 matmuls instead of 8x
            def simple_pass_pair(W, t, u):
                A, B, Bn = W
                ps = []
                xs = []
                for tt in (t, u):
                    c0 = tt * 512
                    pre = mmp.tile([128, 512], F32, tag="mm")
                    pim = mmp.tile([128, 512], F32, tag="mm")
                    ps.append((pre, pim))
                    xs.append((sre[:, c0:c0 + 512], sim[:, c0:c0 + 512]))
                for i in (0, 1):
                    nc.tensor.matmul(ps[i][0][:], (A[:]), xs[i][0], start=True, stop=False)
                    nc.tensor.matmul(ps[i][1][:], (A[:]), xs[i][1], start=True, stop=False)
                for i in (0, 1):
                    nc.tensor.matmul(ps[i][1][:], (B[:]), xs[i][0], start=False, stop=True)
                for i in (0, 1):
                    nc.tensor.matmul(ps[i][0][:], (Bn[:]), xs[i][1], start=False, stop=True)
                for i, tt in enumerate((t, u)):
                    c0 = tt * 512
                    nc.vector.tensor_copy(sre[:, c0:c0 + 512], ps[i][0][:])
                    nc.scalar.copy(out=sim[:, c0:c0 + 512], in_=ps[i][1][:])

            for t in range(0, 32, 2):
                simple_pass_pair(w1t, t, t + 1)

            # ---- T1: transpose partitions (q0..q6) <-> free (q7..q13) ----
            # L1 free = (q7..q13)*128 + (q14..q20); window w = (q14..q20):
            # read col-set {a*128+w}, transpose, write back to same col-set,
            # giving L2: partitions (q7..q13), free = (q0..q6)*128 + (q14..q20).
            for si, s in enumerate((sre, sim)):
                sv = s[:].rearrange("p (a w) -> p a w", w=128)
                for w in range(128):
                    pt = trp.tile([128, 128], SD, tag="tr")
                    nc.tensor.transpose((pt[:]), (sv[:, :, w]), (idt[:]))
                    if (w + si) % 3 == 0:
                        nc.scalar.copy(out=sv[:, :, w], in_=pt[:])
                    else:
                        nc.vector.tensor_copy(sv[:, :, w], pt[:])

            # ---- P2: chunk [6..13]; partitions (q7..q13), q6 = free bit ----
            # L2 free = (q0..q6)*128 + (q14..q20); q6 = bit0 of the outer
            # index => columns alternate 128-blocks by q6.
            sre_v = sre[:].rearrange("p (o q c) -> p o q c", q=2, c=128)
            sim_v = sim[:].rearrange("p (o q c) -> p o q c", q=2, c=128)

            def p2_tile(t):
                o0 = t * 4
                xr = [sre_v[:, o0:o0 + 4, k, :] for k in (0, 1)]
                xi = [sim_v[:, o0:o0 + 4, k, :] for k in (0, 1)]
                ps = []
                for j in (0, 1):
                    pre = mmp.tile([128, 512], F32, tag="mm")
                    pim = mmp.tile([128, 512], F32, tag="mm")
                    cmul_into(pre, pim, w2t[j][0], xr[0], xi[0], start=True)
                    cmul_into(pre, pim, w2t[j][1], xr[1], xi[1], start=False, stop=True)
                    ps.append((pre, pim))
                for j in (0, 1):
                    pre, pim = ps[j]
                    nc.vector.tensor_copy(sre_v[:, o0:o0 + 4, j, :], pre[:])
                    nc.scalar.copy(out=sim_v[:, o0:o0 + 4, j, :], in_=pim[:])

            # ---- T2: transpose partitions (q7..q13) <-> free (q14..q20) ----
            # window o = (q0..q6): read contiguous block [o*128, o*128+128),
            # write back with q13 at stride 64 (free = o*128 + q13*64 +
            # (q7..q12)) so P3's k13-halves are contiguous 64-elem runs.
            # L3: partitions (q14..q20), free as above.
            sv2s = [s[:].rearrange("p (o k c) -> p o k c", k=2, c=64)
                    for s in (sre, sim)]

            def t2_block(o):
                for si, s in enumerate((sre, sim)):
                    pt = trp.tile([128, 128], SD, tag="tr")
                    nc.tensor.transpose(
                        (pt[:]), (s[:, o * 128:o * 128 + 128]), (idt[:]))
                    ptv = pt[:].rearrange("p (c k) -> p k c", k=2)
                    if (o + si) % 3 == 0:
                        nc.scalar.copy(out=sv2s[si][:, o], in_=ptv)
                    else:
                        nc.vector.tensor_copy(sv2s[si][:, o], ptv)

            # ---- P3: chunk [13..20]; partitions (q14..q20), q13 on free ----
            sre_w = sre[:].rearrange("p (o k b) -> p o k b", k=2, b=64)
            sim_w = sim[:].rearrange("p (o k b) -> p o k b", k=2, b=64)

            # staging views: slot = quarter v % 2, each slot 4096 cols
            stg_re_w = stg_re[:].rearrange("p (s o k b) -> p s o k b",
                                           s=2, k=2, b=64)
            stg_im_w = stg_im[:].rearrange("p (s o k b) -> p s o k b",
                                           s=2, k=2, b=64)

            def p3_tile(t):
                o0 = t * 4
                slot = (t & 3) % 2
                so = (t >> 2) * 4  # position of this tile inside its slot
                xr = [sre_w[:, o0:o0 + 4, k, :] for k in (0, 1)]
                xi = [sim_w[:, o0:o0 + 4, k, :] for k in (0, 1)]
                ps = []
                for j in (0, 1):
                    pre = mmp.tile([128, 256], F32, tag="mm")
                    pim = mmp.tile([128, 256], F32, tag="mm")
                    cmul_into(pre, pim, w3t[j][0], xr[0], xi[0], start=True)
                    cmul_into(pre, pim, w3t[j][1], xr[1], xi[1], start=False, stop=True)
                    ps.append((pre, pim))
                for j in (0, 1):
                    pre, pim = ps[j]
                    nc.vector.tensor_copy(stg_re_w[:, slot, so:so + 4, j, :], pre[:])
                    nc.scalar.copy(out=stg_im_w[:, slot, so:so + 4, j, :], in_=pim[:])

            def stage_quarter(v):
                # staging slot (part q14..q20, [b=(q0q1q2), f]) -> a2a_in[v]
                slot = v % 2
                for pl, s in ((0, stg_re), (1, stg_im)):
                    sv = s[:].rearrange("p (t b f) -> p t b f", t=2, b=8)
                    nc.sync.dma_start(
                        out=a2a_in[v][:, pl].rearrange("b p f -> p b f"),
                        in_=sv[:, slot, :, :])
                nc.gpsimd.collective_compute(
                    "AllToAll",
                    mybir.AluOpType.bypass,
                    replica_groups=[list(range(N_CORES))],
                    ins=[a2a_in[v].ap().opt()],
                    outs=[a2a_out[v].ap().opt()],
                )

            # After P3 the f32r state is dead; alias bf16 readback tiles
            # over the same pool slots.  P4 runs as a bf16 matmul directly
            # on the wire-format data (no cast DMAs).
            rb_re = sp.tile([128, 16384], BF16, tag="sre")
            rb_im = sp.tile([128, 16384], BF16, tag="sim")

            def readback_quarter(v):
                # a2a_out[v][s3, pl, (h3,m), f] -> partitions s3*16+m,
                # free = h3*2048 + v*512 + f   (plain bf16 HWDGE DMA)
                for s3 in range(8):
                    for pl, s in ((0, rb_re), (1, rb_im)):
                        sv = (s[s3 * 16:(s3 + 1) * 16, :]
                              .rearrange("m (h3 g f) -> m h3 g f", h3=8, g=4))
                        nc.sync.dma_start(
                            out=sv[:, :, v, :],
                            in_=a2a_out[v][s3, pl]
                                .rearrange("(h3 m) f -> m h3 f", m=16))

            # ---- emission order: pipeline the A2A train under compute ----
            # P2 even tiles cover the o-ranges P3 quarters 0/1 need;
            # odd tiles cover quarters 2/3.
            p3_q = [[t for t in range(32) if (t & 3) == v] for v in range(4)]
            t2_for = lambda ts: [o for t in ts for o in range(4 * t, 4 * t + 4)]

            for t in range(0, 16, 2):
                p2_tile(t)
            for o in t2_for(p3_q[0] + p3_q[1]):
                t2_block(o)
            for t in p3_q[0]:
                p3_tile(t)
            stage_quarter(0)
            for t in p3_q[1]:
                p3_tile(t)
            stage_quarter(1)
            for t in range(1, 16, 2):
                p2_tile(t)
            for o in t2_for(p3_q[2] + p3_q[3]):
                t2_block(o)
            for t in p3_q[2]:
                p3_tile(t)
            stage_quarter(2)
            for t in p3_q[3]:
                p3_tile(t)
            stage_quarter(3)

            # P4 tile t covers free [512t, 512t+512) = fixes
            # (q14q15q16, q3, q4); quarter v = t & 3.  bf16 matmuls on the
            # readback tiles; outputs stream through small f32 tiles.
            def p4_pair(t, u):
                A, B, Bn = w4t
                ps = []
                xs = []
                for tt in (t, u):
                    c0 = tt * 512
                    pre = mmp.tile([128, 512], F32, tag="mm")
                    pim = mmp.tile([128, 512], F32, tag="mm")
                    ps.append((pre, pim))
                    xs.append((rb_re[:, c0:c0 + 512], rb_im[:, c0:c0 + 512]))
                for i in (0, 1):
                    nc.tensor.matmul(ps[i][0][:], A[:], xs[i][0], start=True, stop=False)
                    nc.tensor.matmul(ps[i][1][:], A[:], xs[i][1], start=True, stop=False)
                for i in (0, 1):
                    nc.tensor.matmul(ps[i][1][:], B[:], xs[i][0], start=False, stop=True)
                for i in (0, 1):
                    nc.tensor.matmul(ps[i][0][:], Bn[:], xs[i][1], start=False, stop=True)
                for i, tt in enumerate((t, u)):
                    c0 = tt * 512
                    for pl, ptile in ((0, ps[i][0]), (1, ps[i][1])):
                        ot = op.tile([128, 512], F32, tag="p4out")
                        if pl == 0:
                            nc.vector.tensor_copy(ot[:], ptile[:])
                        else:
                            nc.scalar.copy(out=ot[:], in_=ptile[:])
                        ov = out[pl].rearrange("(p f) -> p f", p=128)
                        nc.sync.dma_start(out=ov[:, c0:c0 + 512], in_=ot[:])

            for v in range(4):
                readback_quarter(v)
                tiles = [t for t in range(32) if (t & 3) == v]
                for i in range(0, 8, 2):
                    p4_pair(tiles[i], tiles[i + 1])

    return nc


# ---------------------------------------------------------------------------
# Host wrapper
# ---------------------------------------------------------------------------

TRACE = False          # set by test harnesses to capture a profile
LAST_EXEC_NS = None
LAST_RESULTS = None


def kernel(state, gates1, gates2):
    global LAST_EXEC_NS, LAST_RESULTS
    state = np.ascontiguousarray(np.asarray(state, dtype=np.float32))
    weights = build_weights(np.asarray(gates1, dtype=np.float32),
                            np.asarray(gates2, dtype=np.float32))

    # shard over (q21,q22,q23) = index mod 8
    shards = np.ascontiguousarray(
        state.reshape(2, 1 << 21, 8).transpose(2, 0, 1))

    nc = build_nc()
    if not nc.is_finalized():
        nc.finalize()
    in_maps = [dict(weights, state=shards[d]) for d in range(N_CORES)]
    res = run_bass_kernel_spmd(nc, in_maps, core_ids=list(range(N_CORES)),
                               trace=TRACE)
    LAST_EXEC_NS = res.exec_time_ns
    LAST_RESULTS = res

    return unshard([res.results[d]["out"] for d in range(N_CORES)])


def unshard(outs):
    # core d holds (q0,q1,q2) = d;
    # out layout = [plane][s=(q21..q23), m=(q17..q20)][h3=(q14..q16)]
    #              [a=(q3..q6)][k=(q13)][c=(q7..q12)]
    full = np.empty((2, 8, 16, 64, 2, 8, 16, 8), dtype=np.float32)
    for d in range(N_CORES):
        od = np.asarray(outs[d]).reshape(2, 8, 16, 8, 16, 2, 64)
        full[:, d] = od.transpose(0, 4, 6, 5, 3, 2, 1)
    return full.reshape(2, 1 << 24)


if __name__ == "__main__":
    rng = np.random.default_rng(0)
    state = rng.standard_normal((2, 1 << 24)).astype(np.float32)
    g1 = rng.standard_normal((24, 2, 2, 2)).astype(np.float32)
    g2 = rng.standard_normal((23, 2, 4, 4)).astype(np.float32)
    out = kernel(state, g1, g2)
    print(out.shape, out.dtype)
